# revision 25
# baseline (speedup 1.0000x reference)
"""Trainium2 Bass kernel for nn_BlocksparseFixedSelfAttention.

Reference computation (B=4, T=2048, EMB=512, KBLK=64):
    Kt = x @ Wk.T + bk ; Qt = x @ Wq.T + bq ; Vt = x @ Wv.T + bv
    head1: block-causal local attention inside each 64-token block
           (row j attends cols [block_start(j) .. j], S = K Q^T)
    head2: row r attends every block start c = 64*i with c <= r
    out = concat(h1, h2) @ Wu.T + bu

Algebraic restructure (zero-bias fast path):
  1. Output projection folded into V. With Wu = [Wu1 | Wu2]:
         out = sum_blk tril(K_b Q_b^T) (V_b Wu1^T) + S2 (V_s Wu2^T) + bu
     so the device computes V1 = x @ (Wv^T Wu1^T) and V2s =
     x_starts @ (Wv^T Wu2^T); the two AV matmuls accumulate into one
     [128, 512] PSUM tile and there is no output GEMM at all.
  2. Score Gram trick: S = K Q^T = x (Wk^T Wq) x^T, so with
     A = Wk^T Wq precomputed on the host, a single projection
     y' = x @ A^T replaces BOTH the K and Q projections; scores are
     dots of y' against the raw (bf16) x already resident in SBUF:
         S1^T[c, r] = sum_f y't[f, c] xt[f, r]
         S2^T[i, r] = sum_f y't[f, start_i] xt[f, r]
  All matmul operands bf16 (host-converted), f32 PSUM accumulate.
  Measured: rel err ~4.6e-3 vs the f32 reference (tol 2e-2).

Per-core PE row budget: y' 16896 + V1 16384 + V2s 2048 + S2 4096 +
S1 4096 + AV 8192 = 51712 moving rows (~21.5 us at 2.4 GHz).

Sharding: data-parallel over (batch, T-half) -> 8 shards, one per core.
Each core gets its 1024 own token rows of x plus the 32 block-start
rows, feature-major (x^T), replicated (pre-folded) weights, and
produces its [1024, 512] slice of the output.

Nonzero biases (never hit by this problem's inputs, which have
fill=zeros biases) fall back to an explicit-K/Q program variant.
"""

import os
import sys

import numpy as np

for _p in ("/opt/trn_rl_repo",):
    if _p not in sys.path and os.path.isdir(_p):
        sys.path.append(_p)

import ml_dtypes

from concourse import bass, bacc, mybir
from concourse import tile
from concourse.bass_utils import run_bass_kernel_spmd

T = 2048
KBLK = 64
EMB = 512
B = 4
NCORES = 8
HALF = T // 2            # tokens owned per core
NSTART = T // KBLK       # 32 block starts
TOT = HALF + NSTART      # own tokens + appended block-start tokens
F32 = mybir.dt.float32
F32R = mybir.dt.float32r
BF16 = mybir.dt.bfloat16
BF16NP = ml_dtypes.bfloat16

NF = EMB // 128          # 4 feature chunks (contraction)
NE = EMB // 128          # 4 embed chunks
NTI = HALF // 128        # 8 own-token tiles


def build_fast():
    """Zero-bias A-form program."""
    nc = bacc.Bacc("TRN2", target_bir_lowering=False, debug=False)

    xt_d = nc.declare_dram_parameter("xt", [EMB, TOT], BF16, False)
    at_d = nc.declare_dram_parameter("at", [EMB, EMB], BF16, False)
    wvu1_d = nc.declare_dram_parameter("wvu1", [EMB, EMB], BF16, False)
    wvu2_d = nc.declare_dram_parameter("wvu2", [EMB, EMB], BF16, False)
    m1_d = nc.declare_dram_parameter("mask1", [128, 128], BF16, False)
    m2_d = nc.declare_dram_parameter("mask2", [NSTART, HALF], BF16, False)
    out_d = nc.declare_dram_parameter("out", [HALF, EMB], BF16, True)

    with tile.TileContext(nc) as tc:
        with (
            tc.tile_pool(name="const", bufs=1) as cpool,
            tc.tile_pool(name="big", bufs=1) as bpool,
            tc.tile_pool(name="work", bufs=3) as wpool,
            tc.tile_pool(name="ps", bufs=8, space="PSUM") as pspool,
        ):
            def psum(tag="ps"):
                return pspool.tile([128, 512], F32, tag=tag, name=tag, bufs=8)

            # ---- PE warmup: ramp the tensor engine's p-state while the
            # first DMAs are in flight (operands come from a memset, so
            # no DMA dependency). ----------------------------------------
            warm_sb = cpool.tile([128, 512], BF16, name="warm_sb")
            nc.vector.memset(warm_sb[:], 1.0)
            for _ in range(6):
                pw = psum()
                nc.tensor.matmul(pw[:, :512], warm_sb[:, :128],
                                 warm_sb[:, :512], start=True, stop=True)

            # ---- DMA: y'-phase operands (at, xt) first. xt and yt are
            # padded from 1056 to 1152 columns with zeros so the V2s/S2
            # stationaries [*, 1024:1152] are full 128-wide tiles — every
            # matmul then uses the same 128x128 PE tile config and no
            # tile-size switch ever blocks LDWEIGHTS prefetch. ------------
            TOTP = TOT + (128 - NSTART)          # 1152
            at_flat = cpool.tile([128, NF * EMB], BF16, name="at_flat")
            at_sb = [at_flat[:, fi * EMB:(fi + 1) * EMB] for fi in range(NF)]
            xt_flat = bpool.tile([128, NF * TOTP], BF16, name="xt_flat")
            xt_sb = [xt_flat[:, fi * TOTP:(fi + 1) * TOTP] for fi in range(NF)]
            for fi in range(NF):
                nc.vector.memset(xt_sb[fi][:, TOT:TOTP], 0.0)
            # first chunks split so the first y' matmul's exact operands
            # (at0 cols 0:128, xt0 cols 0:512) land earliest; at2/at3 ride
            # the otherwise-idle gpsimd queue so the sync ring doesn't
            # serialize all four at transfers
            nc.sync.dma_start(at_sb[0][:, :128], at_d[0:128, :128])
            nc.scalar.dma_start(xt_sb[0][:, :512], xt_d[0:128, :512])
            nc.sync.dma_start(at_sb[0][:, 128:], at_d[0:128, 128:])
            nc.scalar.dma_start(xt_sb[0][:, 512:TOT], xt_d[0:128, 512:])
            nc.gpsimd.dma_start(at_sb[2], at_d[256:384, :])
            nc.sync.dma_start(at_sb[1], at_d[128:256, :])
            nc.scalar.dma_start(xt_sb[1][:, :TOT], xt_d[128:256, :])
            nc.gpsimd.dma_start(at_sb[3], at_d[384:512, :])
            nc.scalar.dma_start(xt_sb[2][:, :TOT], xt_d[256:384, :])
            nc.scalar.dma_start(xt_sb[3][:, :TOT], xt_d[384:512, :])

            def load_w(name, dram, eng):
                t_ = cpool.tile([128, NF * EMB], BF16, name=name)
                chunks = [t_[:, ci * EMB:(ci + 1) * EMB] for ci in range(NF)]
                for ci in range(NF):
                    eng.dma_start(chunks[ci], dram[ci * 128:(ci + 1) * 128, :])
                return chunks

            m2_sb = cpool.tile([NSTART, HALF], BF16, name="m2_sb")
            nc.sync.dma_start(m2_sb[:], m2_d[:])
            m1_sb = cpool.tile([128, 128], BF16, name="m1_sb")
            nc.sync.dma_start(m1_sb[:], m1_d[:])
            wvu2_sb = load_w("wvu2_sb", wvu2_d, nc.scalar)
            wvu1_sb = load_w("wvu1_sb", wvu1_d, nc.scalar)

            # ---- zero-pad tiles: s2t and v2s are padded to 128
            # partitions (rows 32.. stay zero) so the head2 AV matmuls use
            # the same 128x128 PE tile config as everything else — a
            # 32-row stationary forces a tile-size switch that blocks
            # LDWEIGHTS prefetch and costs ~100 ns per matmul ------------
            v2s_sb = cpool.tile([128, EMB], BF16, name="v2s_sb")
            nc.vector.memset(v2s_sb[:], 0.0)
            s2t_sb = bpool.tile([128, HALF], BF16, name="s2t_sb")
            nc.vector.memset(s2t_sb[:], 0.0)

            # ---- y' projection: y't[f1, c] = sum_f2 At[f2, f1] xt[f2, c]
            # (ei, fi) outer with the three t-spans inside: consecutive
            # matmuls share one stationary, so the 32-wide tail's
            # LDWEIGHTS hides under the preceding 512-row matmul. ---------
            yt_sb = [bpool.tile([128, TOTP], BF16, name=f"yt_sb{ei}")
                     for ei in range(NE)]
            for ei in range(NE):
                nc.vector.memset(yt_sb[ei][:, TOT:TOTP], 0.0)
            spans = [(0, 512), (512, 512), (1024, NSTART)]
            for ei in range(NE):
                pss = [psum() for _ in spans]
                for fi in range(NF):
                    for si, (t0, sw) in enumerate(spans):
                        nc.tensor.matmul(
                            pss[si][:, :sw],
                            at_sb[fi][:, ei * 128:(ei + 1) * 128],
                            xt_sb[fi][:, t0:t0 + sw],
                            start=(fi == 0), stop=(fi == NF - 1))
                for si, (t0, sw) in enumerate(spans):
                    nc.vector.tensor_copy(yt_sb[ei][:, t0:t0 + sw],
                                          pss[si][:, :sw])

            # ---- V2s = x_starts @ (Wv^T Wu2^T); stationary is the padded
            # [128]-wide starts block so the PE tile config stays 128x128 -
            ps = psum()
            for fi in range(NF):
                nc.tensor.matmul(ps[:, :], xt_sb[fi][:, HALF:TOTP],
                                 wvu2_sb[fi][:],
                                 start=(fi == 0), stop=(fi == NF - 1))
            nc.scalar.copy(v2s_sb[:NSTART, :], ps[:NSTART, :])

            # ---- head2 scores S2^T[i, r] = y'_starts . x_r, masked ------
            for t0 in (0, 512):
                ps2 = psum()
                for fi in range(NF):
                    nc.tensor.matmul(ps2[:, :512],
                                     yt_sb[fi][:, HALF:TOTP],
                                     xt_sb[fi][:, t0:t0 + 512],
                                     start=(fi == 0), stop=(fi == NF - 1))
                nc.vector.tensor_mul(s2t_sb[:NSTART, t0:t0 + 512],
                                     ps2[:NSTART, :512],
                                     m2_sb[:, t0:t0 + 512])

            # ---- main tile pipeline. Per iteration the PE stream is
            #   V1fi0 s1fi0 V1fi1 [h2AV'] s1fi1 V1fi2 [h1AV'] s1fi2
            #   V1fi3 s1fi3                       (' = tile ti-1)
            # Each short-stream matmul (128-row score, AV) directly
            # follows a 512-row V1 matmul so its LDWEIGHTS prefetches
            # under the long stream and the weight-load port never
            # saturates. The two AV matmuls accumulate into one PSUM
            # bank; scalar drains it to bf16 and sync stores it. ---------
            v1n_sb = [bpool.tile([128, EMB], BF16, name=f"v1n_sb{ti}")
                      for ti in range(NTI)]
            s1ts = [None] * NTI

            def emit_v1_s1(ti):
                t0 = ti * 128
                ps = psum()
                for fi in range(NF):
                    nc.tensor.matmul(ps[:, :], xt_sb[fi][:, t0:t0 + 128],
                                     wvu1_sb[fi][:],
                                     start=(fi == 0), stop=(fi == NF - 1))
                nc.scalar.copy(v1n_sb[ti][:], ps[:, :])
                ps1 = psum()
                for fi in range(NF):
                    nc.tensor.matmul(ps1[:, :128],
                                     yt_sb[fi][:, t0:t0 + 128],
                                     xt_sb[fi][:, t0:t0 + 128],
                                     start=(fi == 0), stop=(fi == NF - 1))
                s1t = wpool.tile([128, 128], BF16, tag="s1t", name="s1t",
                                 bufs=4)
                nc.vector.tensor_mul(s1t[:], ps1[:, :128], m1_sb[:])
                s1ts[ti] = s1t

            def emit_av_out(ti):
                t0 = ti * 128
                ph = psum()
                ot = wpool.tile([128, EMB], BF16, tag="ot", name="ot", bufs=3)
                nc.tensor.matmul(ph[:, :], s1ts[ti][:], v1n_sb[ti][:],
                                 start=True, stop=False)
                nc.tensor.matmul(ph[:, :], s2t_sb[:, t0:t0 + 128],
                                 v2s_sb[:], start=False, stop=True)
                if ti < NTI - 1:
                    nc.scalar.copy(ot[:], ph[:, :])
                    nc.sync.dma_start(out_d[t0:t0 + 128, :], ot[:])
                else:
                    # last tile: split the drain across two engines and two
                    # DMA queues to shorten the serial tail
                    nc.vector.tensor_copy(ot[:, :256], ph[:, :256])
                    nc.sync.dma_start(out_d[t0:t0 + 128, :256], ot[:, :256])
                    nc.scalar.copy(ot[:, 256:], ph[:, 256:])
                    nc.scalar.dma_start(out_d[t0:t0 + 128, 256:],
                                        ot[:, 256:])

            for ti in range(NTI + 1):
                if ti < NTI:
                    emit_v1_s1(ti)
                if ti >= 1:
                    emit_av_out(ti - 1)

    return nc


def build_bias():
    """Generic program with explicit K/Q projections and bias adds
    (correctness fallback; the harness's biases are all zero)."""
    nc = bacc.Bacc("TRN2", target_bir_lowering=False, debug=False)

    xt_d = nc.declare_dram_parameter("xt", [EMB, TOT], BF16, False)
    wkt_d = nc.declare_dram_parameter("wkt", [EMB, EMB], BF16, False)
    wqt_d = nc.declare_dram_parameter("wqt", [EMB, EMB], BF16, False)
    wvu1_d = nc.declare_dram_parameter("wvu1", [EMB, EMB], BF16, False)
    wvu2_d = nc.declare_dram_parameter("wvu2", [EMB, EMB], BF16, False)
    m1_d = nc.declare_dram_parameter("mask1", [128, 128], BF16, False)
    m2_d = nc.declare_dram_parameter("mask2", [NSTART, HALF], BF16, False)
    bkq_d = nc.declare_dram_parameter("bkq", [128, 2 * NE], F32, False)
    bv1_d = nc.declare_dram_parameter("bv1r", [1, EMB], F32, False)
    bv2_d = nc.declare_dram_parameter("bv2r", [1, EMB], F32, False)
    bu_d = nc.declare_dram_parameter("bur", [1, EMB], F32, False)
    ones_d = nc.declare_dram_parameter("ones", [1, 128], F32, False)
    out_d = nc.declare_dram_parameter("out", [HALF, EMB], F32, True)

    with tile.TileContext(nc) as tc:
        with (
            tc.tile_pool(name="const", bufs=1) as cpool,
            tc.tile_pool(name="big", bufs=1) as bpool,
            tc.tile_pool(name="work", bufs=3) as wpool,
            tc.tile_pool(name="ps", bufs=8, space="PSUM") as pspool,
        ):
            def psum(tag="ps"):
                return pspool.tile([128, 512], F32, tag=tag, name=tag, bufs=8)

            wkt_flat = cpool.tile([128, NF * EMB], BF16, name="wkt_flat")
            wkt_sb = [wkt_flat[:, fi * EMB:(fi + 1) * EMB] for fi in range(NF)]
            xt_flat = bpool.tile([128, NF * TOT], BF16, name="xt_flat")
            xt_sb = [xt_flat[:, fi * TOT:(fi + 1) * TOT] for fi in range(NF)]
            for fi in range(NF):
                nc.sync.dma_start(wkt_sb[fi], wkt_d[fi * 128:(fi + 1) * 128, :])
                nc.scalar.dma_start(xt_sb[fi], xt_d[fi * 128:(fi + 1) * 128, :])

            def load_w(name, dram, eng):
                t_ = cpool.tile([128, NF * EMB], BF16, name=name)
                chunks = [t_[:, ci * EMB:(ci + 1) * EMB] for ci in range(NF)]
                for ci in range(NF):
                    eng.dma_start(chunks[ci], dram[ci * 128:(ci + 1) * 128, :])
                return chunks

            wqt_sb = load_w("wqt_sb", wqt_d, nc.sync)
            bkq_sb = cpool.tile([128, 2 * NE], F32, name="bkq_sb")
            nc.sync.dma_start(bkq_sb[:], bkq_d[:])
            bkc_sb = bkq_sb[:, 0:NE]
            bqc_sb = bkq_sb[:, NE:2 * NE]
            m2_sb = cpool.tile([NSTART, HALF], BF16, name="m2_sb")
            nc.sync.dma_start(m2_sb[:], m2_d[:])
            m1_sb = cpool.tile([128, 128], BF16, name="m1_sb")
            nc.sync.dma_start(m1_sb[:], m1_d[:])
            ones_sb = cpool.tile([1, 128], F32R, name="ones_sb")
            nc.sync.dma_start(ones_sb[:], ones_d[:].bitcast(F32R))
            bv1r_sb = cpool.tile([1, EMB], F32R, name="bv1r_sb")
            nc.sync.dma_start(bv1r_sb[:], bv1_d[:].bitcast(F32R))
            bv2r_sb = cpool.tile([1, EMB], F32R, name="bv2r_sb")
            nc.sync.dma_start(bv2r_sb[:], bv2_d[:].bitcast(F32R))
            bur_sb = cpool.tile([1, EMB], F32R, name="bur_sb")
            nc.sync.dma_start(bur_sb[:], bu_d[:].bitcast(F32R))
            wvu1_sb = load_w("wvu1_sb", wvu1_d, nc.scalar)
            wvu2_sb = load_w("wvu2_sb", wvu2_d, nc.scalar)

            kt_sb = [bpool.tile([128, HALF], BF16, name=f"kt_sb{ei}")
                     for ei in range(NE)]
            for t0 in (0, 512):
                pss = [psum() for _ in range(NE)]
                for fi in range(NF):
                    for ei in range(NE):
                        nc.tensor.matmul(
                            pss[ei][:, :512],
                            wkt_sb[fi][:, ei * 128:(ei + 1) * 128],
                            xt_sb[fi][:, t0:t0 + 512],
                            start=(fi == 0), stop=(fi == NF - 1))
                for ei in range(NE):
                    nc.vector.tensor_scalar_add(
                        kt_sb[ei][:, t0:t0 + 512], pss[ei][:, :512],
                        bkc_sb[:, ei:ei + 1])

            bcast = {}
            for nm, src in (("bv1", bv1r_sb), ("bv2", bv2r_sb),
                            ("bu", bur_sb)):
                pb = psum()
                nc.tensor.matmul(pb[:, :EMB], ones_sb[:1, :], src[:1, :],
                                 start=True, stop=True)
                bb = cpool.tile([128, EMB], F32, name=f"{nm}b_sb")
                nc.vector.tensor_copy(bb[:], pb[:, :EMB])
                bcast[nm] = bb

            qt_sb = [bpool.tile([128, TOT], BF16, name=f"qt_sb{ei}")
                     for ei in range(NE)]
            spans = [(0, 512), (512, 512), (1024, NSTART)]
            for ei in range(NE):
                pss = [psum() for _ in spans]
                for fi in range(NF):
                    for si, (t0, sw) in enumerate(spans):
                        nc.tensor.matmul(
                            pss[si][:, :sw],
                            wqt_sb[fi][:, ei * 128:(ei + 1) * 128],
                            xt_sb[fi][:, t0:t0 + sw],
                            start=(fi == 0), stop=(fi == NF - 1))
                for si, (t0, sw) in enumerate(spans):
                    nc.vector.tensor_scalar_add(
                        qt_sb[ei][:, t0:t0 + sw], pss[si][:, :sw],
                        bqc_sb[:, ei:ei + 1])

            ps = psum()
            for fi in range(NF):
                nc.tensor.matmul(ps[:NSTART, :], xt_sb[fi][:, HALF:TOT],
                                 wvu2_sb[fi][:],
                                 start=(fi == 0), stop=(fi == NF - 1))
            v2s_sb = cpool.tile([NSTART, EMB], BF16, name="v2s_sb")
            nc.vector.tensor_add(v2s_sb[:], ps[:NSTART, :],
                                 bcast["bv2"][:NSTART, :])

            s2t_sb = bpool.tile([NSTART, HALF], BF16, name="s2t_sb")
            for t0 in (0, 512):
                ps2 = psum()
                for ei in range(NE):
                    nc.tensor.matmul(ps2[:NSTART, :512],
                                     qt_sb[ei][:, HALF:TOT],
                                     kt_sb[ei][:, t0:t0 + 512],
                                     start=(ei == 0), stop=(ei == NE - 1))
                nc.vector.tensor_mul(s2t_sb[:, t0:t0 + 512], ps2[:NSTART, :512],
                                     m2_sb[:, t0:t0 + 512])

            v1n_sb = [bpool.tile([128, EMB], BF16, name=f"v1n_sb{ti}")
                      for ti in range(NTI)]
            s1ts = [None] * NTI

            def emit_v1_s1(ti):
                t0 = ti * 128
                ps = psum()
                for fi in range(NF):
                    nc.tensor.matmul(ps[:, :], xt_sb[fi][:, t0:t0 + 128],
                                     wvu1_sb[fi][:],
                                     start=(fi == 0), stop=(fi == NF - 1))
                nc.vector.tensor_add(v1n_sb[ti][:], ps[:, :], bcast["bv1"][:])
                ps1 = psum()
                for ei in range(NE):
                    nc.tensor.matmul(ps1[:, :128],
                                     qt_sb[ei][:, t0:t0 + 128],
                                     kt_sb[ei][:, t0:t0 + 128],
                                     start=(ei == 0), stop=(ei == NE - 1))
                s1t = wpool.tile([128, 128], BF16, tag="s1t", name="s1t",
                                 bufs=4)
                nc.vector.tensor_mul(s1t[:], ps1[:, :128], m1_sb[:])
                s1ts[ti] = s1t

            def emit_av_out(ti):
                t0 = ti * 128
                ph = psum()
                nc.tensor.matmul(ph[:, :], s1ts[ti][:], v1n_sb[ti][:],
                                 start=True, stop=False)
                nc.tensor.matmul(ph[:, :], s2t_sb[:, t0:t0 + 128],
                                 v2s_sb[:], start=False, stop=True)
                ot = wpool.tile([128, EMB], F32, tag="ot", name="ot", bufs=3)
                nc.vector.tensor_add(ot[:], ph[:, :], bcast["bu"][:])
                nc.sync.dma_start(out_d[t0:t0 + 128, :], ot[:])

            for ti in range(NTI + 1):
                if ti < NTI:
                    emit_v1_s1(ti)
                if ti >= 1:
                    emit_av_out(ti - 1)

    return nc


_NC_CACHE = {}


def _get_program(with_bias):
    if with_bias not in _NC_CACHE:
        nc = build_bias() if with_bias else build_fast()
        nc.compile()          # bacc passes: wait splitting, reg alloc, ISA
        _NC_CACHE[with_bias] = nc
    return _NC_CACHE[with_bias]


def _make_masks():
    tri = np.triu(np.ones((KBLK, KBLK), np.float32))           # [c_l, r_l]
    m1 = np.kron(np.eye(2, dtype=np.float32), tri)             # [128, 128]
    # mask2[h][i, rl] = 1 if 64*i <= h*HALF + rl
    r = np.arange(HALF)
    m2 = []
    for h in range(2):
        blk = (h * HALF + r) // KBLK                           # [HALF]
        m2.append((np.arange(NSTART)[:, None] <= blk[None, :])
                  .astype(np.float32))
    return m1, m2


def make_in_maps(inputs, with_bias):
    x = np.asarray(inputs["x"], np.float32)
    wk = np.asarray(inputs["Wk"], np.float32)
    wq = np.asarray(inputs["Wq"], np.float32)
    wv = np.asarray(inputs["Wv"], np.float32)
    wu = np.asarray(inputs["Wu"], np.float32)
    bk = np.asarray(inputs["bk"], np.float32)
    bq = np.asarray(inputs["bq"], np.float32)
    bv = np.asarray(inputs["bv"], np.float32)
    bu = np.asarray(inputs["bu"], np.float32)

    # host-side weight preprocessing (pure weight-product folding)
    wvu1 = np.ascontiguousarray(wv.T @ wu[:, :EMB].T).astype(BF16NP)
    wvu2 = np.ascontiguousarray(wv.T @ wu[:, EMB:].T).astype(BF16NP)

    m1, m2 = _make_masks()
    starts = np.arange(NSTART) * KBLK

    in_maps = []
    for c in range(NCORES):
        b, h = c // 2, c % 2
        xin = np.concatenate(
            [x[b, h * HALF:(h + 1) * HALF], x[b, starts]], axis=0)
        m = {
            "xt": np.ascontiguousarray(xin.T.astype(BF16NP)),
            "wvu1": wvu1, "wvu2": wvu2,
            "mask1": m1.astype(BF16NP), "mask2": m2[h].astype(BF16NP),
        }
        if with_bias:
            m.update({
                "wkt": np.ascontiguousarray(wk.T).astype(BF16NP),
                "wqt": np.ascontiguousarray(wq.T).astype(BF16NP),
                "bkq": np.ascontiguousarray(np.concatenate(
                    [bk.reshape(EMB // 128, 128).T,
                     bq.reshape(EMB // 128, 128).T], axis=1)),
                "bv1r": (bv @ wu[:, :EMB].T).reshape(1, EMB).copy(),
                "bv2r": (bv @ wu[:, EMB:].T).reshape(1, EMB).copy(),
                "bur": bu.reshape(1, EMB).copy(),
                "ones": np.ones((1, 128), np.float32),
            })
        else:
            # At[f2, f1] = (Wk^T Wq)^T = Wq^T Wk
            m["at"] = np.ascontiguousarray(wq.T @ wk).astype(BF16NP)
        in_maps.append(m)
    return in_maps


def _ensure_ntff_hook():
    """The agent image lacks antenv.axon_hooks; synthesize it and register
    the ctypes NTFF profiling hook so trace=True works under axon."""
    import importlib.util
    if importlib.util.find_spec("antenv.axon_hooks") is not None:
        return
    import types
    import antenv
    m = types.ModuleType("antenv.axon_hooks")
    m._hook = None
    def set_axon_ntff_profile_hook(h):
        m._hook = h
    def get_axon_ntff_profile_hook():
        return m._hook
    m.set_axon_ntff_profile_hook = set_axon_ntff_profile_hook
    m.get_axon_ntff_profile_hook = get_axon_ntff_profile_hook
    sys.modules["antenv.axon_hooks"] = m
    antenv.axon_hooks = m
    try:
        from trn_agent_boot.trn_boot import _ntff_profile_via_ctypes
        m._hook = _ntff_profile_via_ctypes("/opt/axon/libaxon_pjrt.so")
    except Exception:
        pass


def run_sharded(inputs, trace=False, trace_kwargs=None):
    """inputs: dict of full numpy arrays keyed like setup_inputs().
    Returns (full_output [B, T, EMB] float32, BassKernelResults)."""
    if trace:
        _ensure_ntff_hook()
    with_bias = any(
        float(np.abs(np.asarray(inputs[k])).max()) != 0.0
        for k in ("bk", "bq", "bv", "bu"))
    in_maps = make_in_maps(inputs, with_bias)
    nc = _get_program(with_bias)
    res = run_bass_kernel_spmd(nc, in_maps, list(range(NCORES)), trace=trace,
                               **(trace_kwargs or {}))

    out = np.empty((B, T, EMB), np.float32)
    for c in range(NCORES):
        b, h = c // 2, c % 2
        out[b, h * HALF:(h + 1) * HALF] = res.results[c]["out"]
    return out, res


def kernel(**inputs):
    out, _ = run_sharded(inputs, trace=False)
    return out


# revision 26
# speedup vs baseline: 1.1251x; 1.1251x over previous
"""Trainium2 Bass kernel for nn_BlocksparseFixedSelfAttention.

Reference computation (B=4, T=2048, EMB=512, KBLK=64):
    Kt = x @ Wk.T + bk ; Qt = x @ Wq.T + bq ; Vt = x @ Wv.T + bv
    head1: block-causal local attention inside each 64-token block
           (row j attends cols [block_start(j) .. j], S = K Q^T)
    head2: row r attends every block start c = 64*i with c <= r
    out = concat(h1, h2) @ Wu.T + bu

Algebraic restructure (zero-bias fast path):
  1. Output projection folded into V. With Wu = [Wu1 | Wu2]:
         out = sum_blk tril(K_b Q_b^T) (V_b Wu1^T) + S2 (V_s Wu2^T) + bu
     so the device computes V1 = x @ (Wv^T Wu1^T) and V2s =
     x_starts @ (Wv^T Wu2^T); the two AV matmuls accumulate into one
     [128, 512] PSUM tile and there is no output GEMM at all.
  2. Score Gram trick: S = K Q^T = x (Wk^T Wq) x^T, so with
     A = Wk^T Wq precomputed on the host, a single projection
     y' = x @ A^T replaces BOTH the K and Q projections; scores are
     dots of y' against the raw (bf16) x already resident in SBUF:
         S1^T[c, r] = sum_f y't[f, c] xt[f, r]
         S2^T[i, r] = sum_f y't[f, start_i] xt[f, r]
  All matmul operands bf16 (host-converted), f32 PSUM accumulate.
  Measured: rel err ~4.6e-3 vs the f32 reference (tol 2e-2).

Per-core PE row budget: y' 16896 + V1 16384 + V2s 2048 + S2 4096 +
S1 4096 + AV 8192 = 51712 moving rows (~21.5 us at 2.4 GHz).

Sharding: data-parallel over (batch, T-half) -> 8 shards, one per core.
Each core gets its 1024 own token rows of x plus the 32 block-start
rows, feature-major (x^T), replicated (pre-folded) weights, and
produces its [1024, 512] slice of the output.

Nonzero biases (never hit by this problem's inputs, which have
fill=zeros biases) fall back to an explicit-K/Q program variant.
"""

import os
import sys

import numpy as np

for _p in ("/opt/trn_rl_repo",):
    if _p not in sys.path and os.path.isdir(_p):
        sys.path.append(_p)

import ml_dtypes

from concourse import bass, bacc, mybir
from concourse import tile
from concourse.bass_utils import run_bass_kernel_spmd

T = 2048
KBLK = 64
EMB = 512
B = 4
NCORES = 8
HALF = T // 2            # tokens owned per core
NSTART = T // KBLK       # 32 block starts
TOT = HALF + NSTART      # own tokens + appended block-start tokens
F32 = mybir.dt.float32
F32R = mybir.dt.float32r
BF16 = mybir.dt.bfloat16
BF16NP = ml_dtypes.bfloat16

NF = EMB // 128          # 4 feature chunks (contraction)
NE = EMB // 128          # 4 embed chunks
NTI = HALF // 128        # 8 own-token tiles


def build_fast():
    """Zero-bias A-form program."""
    nc = bacc.Bacc("TRN2", target_bir_lowering=False, debug=False)

    xt_d = nc.declare_dram_parameter("xt", [EMB, TOT], BF16, False)
    at_d = nc.declare_dram_parameter("at", [EMB, EMB], BF16, False)
    wvu1_d = nc.declare_dram_parameter("wvu1", [EMB, EMB], BF16, False)
    wvu2_d = nc.declare_dram_parameter("wvu2", [EMB, EMB], BF16, False)
    m1_d = nc.declare_dram_parameter("mask1", [128, 128], BF16, False)
    m2_d = nc.declare_dram_parameter("mask2", [NSTART, HALF], BF16, False)
    out_d = nc.declare_dram_parameter("out", [HALF, EMB], BF16, True)

    with tile.TileContext(nc) as tc:
        with (
            tc.tile_pool(name="const", bufs=1) as cpool,
            tc.tile_pool(name="big", bufs=1) as bpool,
            tc.tile_pool(name="work", bufs=3) as wpool,
            tc.tile_pool(name="ps", bufs=8, space="PSUM") as pspool,
        ):
            def psum(tag="ps"):
                return pspool.tile([128, 512], F32, tag=tag, name=tag, bufs=8)

            # ---- PE warmup: ramp the tensor engine's p-state while the
            # first DMAs are in flight (operands come from a memset, so
            # no DMA dependency). ----------------------------------------
            warm_sb = cpool.tile([128, 512], BF16, name="warm_sb")
            nc.vector.memset(warm_sb[:], 1.0)
            for _ in range(6):
                pw = psum()
                nc.tensor.matmul(pw[:, :512], warm_sb[:, :128],
                                 warm_sb[:, :512], start=True, stop=True)

            # ---- DMA: y'-phase operands (at, xt) first. xt and yt are
            # padded from 1056 to 1152 columns with zeros so the V2s/S2
            # stationaries [*, 1024:1152] are full 128-wide tiles — every
            # matmul then uses the same 128x128 PE tile config and no
            # tile-size switch ever blocks LDWEIGHTS prefetch. ------------
            TOTP = TOT + (128 - NSTART)          # 1152
            at_flat = cpool.tile([128, NF * EMB], BF16, name="at_flat")
            at_sb = [at_flat[:, fi * EMB:(fi + 1) * EMB] for fi in range(NF)]
            xt_flat = bpool.tile([128, NF * TOTP], BF16, name="xt_flat")
            xt_sb = [xt_flat[:, fi * TOTP:(fi + 1) * TOTP] for fi in range(NF)]
            for fi in range(NF):
                nc.vector.memset(xt_sb[fi][:, TOT:TOTP], 0.0)
            # first chunks split so the first y' matmul's exact operands
            # (at0 cols 0:128, xt0 cols 0:512) land earliest; at2/at3 ride
            # the otherwise-idle gpsimd queue so the sync ring doesn't
            # serialize all four at transfers
            nc.sync.dma_start(at_sb[0][:, :128], at_d[0:128, :128])
            nc.scalar.dma_start(xt_sb[0][:, :512], xt_d[0:128, :512])
            nc.sync.dma_start(at_sb[0][:, 128:], at_d[0:128, 128:])
            nc.scalar.dma_start(xt_sb[0][:, 512:TOT], xt_d[0:128, 512:])
            nc.gpsimd.dma_start(at_sb[2], at_d[256:384, :])
            nc.sync.dma_start(at_sb[1], at_d[128:256, :])
            nc.scalar.dma_start(xt_sb[1][:, :TOT], xt_d[128:256, :])
            nc.gpsimd.dma_start(at_sb[3], at_d[384:512, :])
            nc.scalar.dma_start(xt_sb[2][:, :TOT], xt_d[256:384, :])
            nc.scalar.dma_start(xt_sb[3][:, :TOT], xt_d[384:512, :])

            def load_w(name, dram, eng):
                t_ = cpool.tile([128, NF * EMB], BF16, name=name)
                chunks = [t_[:, ci * EMB:(ci + 1) * EMB] for ci in range(NF)]
                for ci in range(NF):
                    eng.dma_start(chunks[ci], dram[ci * 128:(ci + 1) * 128, :])
                return chunks

            m2_sb = cpool.tile([NSTART, HALF], BF16, name="m2_sb")
            nc.sync.dma_start(m2_sb[:], m2_d[:])
            m1_sb = cpool.tile([128, 128], BF16, name="m1_sb")
            nc.sync.dma_start(m1_sb[:], m1_d[:])
            wvu2_sb = load_w("wvu2_sb", wvu2_d, nc.scalar)
            wvu1_sb = load_w("wvu1_sb", wvu1_d, nc.scalar)

            # ---- zero-pad tiles: s2t and v2s are padded to 128
            # partitions (rows 32.. stay zero) so the head2 AV matmuls use
            # the same 128x128 PE tile config as everything else — a
            # 32-row stationary forces a tile-size switch that blocks
            # LDWEIGHTS prefetch and costs ~100 ns per matmul ------------
            v2s_sb = cpool.tile([128, EMB], BF16, name="v2s_sb")
            nc.vector.memset(v2s_sb[:], 0.0)
            s2t_sb = bpool.tile([128, HALF], BF16, name="s2t_sb")
            nc.vector.memset(s2t_sb[:], 0.0)

            # ---- y' projection: y't[f1, c] = sum_f2 At[f2, f1] xt[f2, c]
            # (ei, fi) outer with the three t-spans inside: consecutive
            # matmuls share one stationary, so the 32-wide tail's
            # LDWEIGHTS hides under the preceding 512-row matmul. ---------
            yt_sb = [bpool.tile([128, TOTP], BF16, name=f"yt_sb{ei}")
                     for ei in range(NE)]
            for ei in range(NE):
                nc.vector.memset(yt_sb[ei][:, TOT:TOTP], 0.0)
            spans = [(0, 512), (512, 512), (1024, NSTART)]
            for ei in range(NE):
                pss = [psum() for _ in spans]
                for fi in range(NF):
                    for si, (t0, sw) in enumerate(spans):
                        nc.tensor.matmul(
                            pss[si][:, :sw],
                            at_sb[fi][:, ei * 128:(ei + 1) * 128],
                            xt_sb[fi][:, t0:t0 + sw],
                            start=(fi == 0), stop=(fi == NF - 1))
                for si, (t0, sw) in enumerate(spans):
                    nc.vector.tensor_copy(yt_sb[ei][:, t0:t0 + sw],
                                          pss[si][:, :sw])

            # ---- V2s = x_starts @ (Wv^T Wu2^T); stationary is the padded
            # [128]-wide starts block so the PE tile config stays 128x128 -
            ps = psum()
            for fi in range(NF):
                nc.tensor.matmul(ps[:, :], xt_sb[fi][:, HALF:TOTP],
                                 wvu2_sb[fi][:],
                                 start=(fi == 0), stop=(fi == NF - 1))
            nc.scalar.copy(v2s_sb[:NSTART, :], ps[:NSTART, :])

            # ---- head2 scores S2^T[i, r] = y'_starts . x_r, masked ------
            for t0 in (0, 512):
                ps2 = psum()
                for fi in range(NF):
                    nc.tensor.matmul(ps2[:, :512],
                                     yt_sb[fi][:, HALF:TOTP],
                                     xt_sb[fi][:, t0:t0 + 512],
                                     start=(fi == 0), stop=(fi == NF - 1))
                nc.vector.tensor_mul(s2t_sb[:NSTART, t0:t0 + 512],
                                     ps2[:NSTART, :512],
                                     m2_sb[:, t0:t0 + 512])

            # ---- main tile pipeline. Per iteration the PE stream is
            #   V1fi0 s1fi0 V1fi1 [h2AV'] s1fi1 V1fi2 [h1AV'] s1fi2
            #   V1fi3 s1fi3                       (' = tile ti-1)
            # Each short-stream matmul (128-row score, AV) directly
            # follows a 512-row V1 matmul so its LDWEIGHTS prefetches
            # under the long stream and the weight-load port never
            # saturates. The two AV matmuls accumulate into one PSUM
            # bank; scalar drains it to bf16 and sync stores it. ---------
            v1n_sb = [bpool.tile([128, EMB], BF16, name=f"v1n_sb{ti}")
                      for ti in range(NTI)]
            s1ts = [None] * NTI

            def emit_v1_s1(ti):
                t0 = ti * 128
                ps = psum()
                for fi in range(NF):
                    nc.tensor.matmul(ps[:, :], xt_sb[fi][:, t0:t0 + 128],
                                     wvu1_sb[fi][:],
                                     start=(fi == 0), stop=(fi == NF - 1))
                nc.scalar.copy(v1n_sb[ti][:], ps[:, :])
                ps1 = psum()
                for fi in range(NF):
                    nc.tensor.matmul(ps1[:, :128],
                                     yt_sb[fi][:, t0:t0 + 128],
                                     xt_sb[fi][:, t0:t0 + 128],
                                     start=(fi == 0), stop=(fi == NF - 1))
                s1t = wpool.tile([128, 128], BF16, tag="s1t", name="s1t",
                                 bufs=4)
                nc.vector.tensor_mul(s1t[:], ps1[:, :128], m1_sb[:])
                s1ts[ti] = s1t

            def emit_av_out(ti):
                t0 = ti * 128
                ph = psum()
                ot = wpool.tile([128, EMB], BF16, tag="ot", name="ot", bufs=3)
                nc.tensor.matmul(ph[:, :], s1ts[ti][:], v1n_sb[ti][:],
                                 start=True, stop=False)
                nc.tensor.matmul(ph[:, :], s2t_sb[:, t0:t0 + 128],
                                 v2s_sb[:], start=False, stop=True)
                if ti < NTI - 1:
                    nc.scalar.copy(ot[:], ph[:, :])
                    nc.sync.dma_start(out_d[t0:t0 + 128, :], ot[:])
                else:
                    # last tile: split the drain across two engines and two
                    # DMA queues to shorten the serial tail
                    nc.vector.tensor_copy(ot[:, :256], ph[:, :256])
                    nc.sync.dma_start(out_d[t0:t0 + 128, :256], ot[:, :256])
                    nc.scalar.copy(ot[:, 256:], ph[:, 256:])
                    nc.scalar.dma_start(out_d[t0:t0 + 128, 256:],
                                        ot[:, 256:])

            for ti in range(NTI + 1):
                if ti < NTI:
                    emit_v1_s1(ti)
                if ti >= 1:
                    emit_av_out(ti - 1)

    return nc


def build_bias():
    """Generic program with explicit K/Q projections and bias adds
    (correctness fallback; the harness's biases are all zero)."""
    nc = bacc.Bacc("TRN2", target_bir_lowering=False, debug=False)

    xt_d = nc.declare_dram_parameter("xt", [EMB, TOT], BF16, False)
    wkt_d = nc.declare_dram_parameter("wkt", [EMB, EMB], BF16, False)
    wqt_d = nc.declare_dram_parameter("wqt", [EMB, EMB], BF16, False)
    wvu1_d = nc.declare_dram_parameter("wvu1", [EMB, EMB], BF16, False)
    wvu2_d = nc.declare_dram_parameter("wvu2", [EMB, EMB], BF16, False)
    m1_d = nc.declare_dram_parameter("mask1", [128, 128], BF16, False)
    m2_d = nc.declare_dram_parameter("mask2", [NSTART, HALF], BF16, False)
    bkq_d = nc.declare_dram_parameter("bkq", [128, 2 * NE], F32, False)
    bv1_d = nc.declare_dram_parameter("bv1r", [1, EMB], F32, False)
    bv2_d = nc.declare_dram_parameter("bv2r", [1, EMB], F32, False)
    bu_d = nc.declare_dram_parameter("bur", [1, EMB], F32, False)
    ones_d = nc.declare_dram_parameter("ones", [1, 128], F32, False)
    out_d = nc.declare_dram_parameter("out", [HALF, EMB], F32, True)

    with tile.TileContext(nc) as tc:
        with (
            tc.tile_pool(name="const", bufs=1) as cpool,
            tc.tile_pool(name="big", bufs=1) as bpool,
            tc.tile_pool(name="work", bufs=3) as wpool,
            tc.tile_pool(name="ps", bufs=8, space="PSUM") as pspool,
        ):
            def psum(tag="ps"):
                return pspool.tile([128, 512], F32, tag=tag, name=tag, bufs=8)

            wkt_flat = cpool.tile([128, NF * EMB], BF16, name="wkt_flat")
            wkt_sb = [wkt_flat[:, fi * EMB:(fi + 1) * EMB] for fi in range(NF)]
            xt_flat = bpool.tile([128, NF * TOT], BF16, name="xt_flat")
            xt_sb = [xt_flat[:, fi * TOT:(fi + 1) * TOT] for fi in range(NF)]
            for fi in range(NF):
                nc.sync.dma_start(wkt_sb[fi], wkt_d[fi * 128:(fi + 1) * 128, :])
                nc.scalar.dma_start(xt_sb[fi], xt_d[fi * 128:(fi + 1) * 128, :])

            def load_w(name, dram, eng):
                t_ = cpool.tile([128, NF * EMB], BF16, name=name)
                chunks = [t_[:, ci * EMB:(ci + 1) * EMB] for ci in range(NF)]
                for ci in range(NF):
                    eng.dma_start(chunks[ci], dram[ci * 128:(ci + 1) * 128, :])
                return chunks

            wqt_sb = load_w("wqt_sb", wqt_d, nc.sync)
            bkq_sb = cpool.tile([128, 2 * NE], F32, name="bkq_sb")
            nc.sync.dma_start(bkq_sb[:], bkq_d[:])
            bkc_sb = bkq_sb[:, 0:NE]
            bqc_sb = bkq_sb[:, NE:2 * NE]
            m2_sb = cpool.tile([NSTART, HALF], BF16, name="m2_sb")
            nc.sync.dma_start(m2_sb[:], m2_d[:])
            m1_sb = cpool.tile([128, 128], BF16, name="m1_sb")
            nc.sync.dma_start(m1_sb[:], m1_d[:])
            ones_sb = cpool.tile([1, 128], F32R, name="ones_sb")
            nc.sync.dma_start(ones_sb[:], ones_d[:].bitcast(F32R))
            bv1r_sb = cpool.tile([1, EMB], F32R, name="bv1r_sb")
            nc.sync.dma_start(bv1r_sb[:], bv1_d[:].bitcast(F32R))
            bv2r_sb = cpool.tile([1, EMB], F32R, name="bv2r_sb")
            nc.sync.dma_start(bv2r_sb[:], bv2_d[:].bitcast(F32R))
            bur_sb = cpool.tile([1, EMB], F32R, name="bur_sb")
            nc.sync.dma_start(bur_sb[:], bu_d[:].bitcast(F32R))
            wvu1_sb = load_w("wvu1_sb", wvu1_d, nc.scalar)
            wvu2_sb = load_w("wvu2_sb", wvu2_d, nc.scalar)

            kt_sb = [bpool.tile([128, HALF], BF16, name=f"kt_sb{ei}")
                     for ei in range(NE)]
            for t0 in (0, 512):
                pss = [psum() for _ in range(NE)]
                for fi in range(NF):
                    for ei in range(NE):
                        nc.tensor.matmul(
                            pss[ei][:, :512],
                            wkt_sb[fi][:, ei * 128:(ei + 1) * 128],
                            xt_sb[fi][:, t0:t0 + 512],
                            start=(fi == 0), stop=(fi == NF - 1))
                for ei in range(NE):
                    nc.vector.tensor_scalar_add(
                        kt_sb[ei][:, t0:t0 + 512], pss[ei][:, :512],
                        bkc_sb[:, ei:ei + 1])

            bcast = {}
            for nm, src in (("bv1", bv1r_sb), ("bv2", bv2r_sb),
                            ("bu", bur_sb)):
                pb = psum()
                nc.tensor.matmul(pb[:, :EMB], ones_sb[:1, :], src[:1, :],
                                 start=True, stop=True)
                bb = cpool.tile([128, EMB], F32, name=f"{nm}b_sb")
                nc.vector.tensor_copy(bb[:], pb[:, :EMB])
                bcast[nm] = bb

            qt_sb = [bpool.tile([128, TOT], BF16, name=f"qt_sb{ei}")
                     for ei in range(NE)]
            spans = [(0, 512), (512, 512), (1024, NSTART)]
            for ei in range(NE):
                pss = [psum() for _ in spans]
                for fi in range(NF):
                    for si, (t0, sw) in enumerate(spans):
                        nc.tensor.matmul(
                            pss[si][:, :sw],
                            wqt_sb[fi][:, ei * 128:(ei + 1) * 128],
                            xt_sb[fi][:, t0:t0 + sw],
                            start=(fi == 0), stop=(fi == NF - 1))
                for si, (t0, sw) in enumerate(spans):
                    nc.vector.tensor_scalar_add(
                        qt_sb[ei][:, t0:t0 + sw], pss[si][:, :sw],
                        bqc_sb[:, ei:ei + 1])

            ps = psum()
            for fi in range(NF):
                nc.tensor.matmul(ps[:NSTART, :], xt_sb[fi][:, HALF:TOT],
                                 wvu2_sb[fi][:],
                                 start=(fi == 0), stop=(fi == NF - 1))
            v2s_sb = cpool.tile([NSTART, EMB], BF16, name="v2s_sb")
            nc.vector.tensor_add(v2s_sb[:], ps[:NSTART, :],
                                 bcast["bv2"][:NSTART, :])

            s2t_sb = bpool.tile([NSTART, HALF], BF16, name="s2t_sb")
            for t0 in (0, 512):
                ps2 = psum()
                for ei in range(NE):
                    nc.tensor.matmul(ps2[:NSTART, :512],
                                     qt_sb[ei][:, HALF:TOT],
                                     kt_sb[ei][:, t0:t0 + 512],
                                     start=(ei == 0), stop=(ei == NE - 1))
                nc.vector.tensor_mul(s2t_sb[:, t0:t0 + 512], ps2[:NSTART, :512],
                                     m2_sb[:, t0:t0 + 512])

            v1n_sb = [bpool.tile([128, EMB], BF16, name=f"v1n_sb{ti}")
                      for ti in range(NTI)]
            s1ts = [None] * NTI

            def emit_v1_s1(ti):
                t0 = ti * 128
                ps = psum()
                for fi in range(NF):
                    nc.tensor.matmul(ps[:, :], xt_sb[fi][:, t0:t0 + 128],
                                     wvu1_sb[fi][:],
                                     start=(fi == 0), stop=(fi == NF - 1))
                nc.vector.tensor_add(v1n_sb[ti][:], ps[:, :], bcast["bv1"][:])
                ps1 = psum()
                for ei in range(NE):
                    nc.tensor.matmul(ps1[:, :128],
                                     qt_sb[ei][:, t0:t0 + 128],
                                     kt_sb[ei][:, t0:t0 + 128],
                                     start=(ei == 0), stop=(ei == NE - 1))
                s1t = wpool.tile([128, 128], BF16, tag="s1t", name="s1t",
                                 bufs=4)
                nc.vector.tensor_mul(s1t[:], ps1[:, :128], m1_sb[:])
                s1ts[ti] = s1t

            def emit_av_out(ti):
                t0 = ti * 128
                ph = psum()
                nc.tensor.matmul(ph[:, :], s1ts[ti][:], v1n_sb[ti][:],
                                 start=True, stop=False)
                nc.tensor.matmul(ph[:, :], s2t_sb[:, t0:t0 + 128],
                                 v2s_sb[:], start=False, stop=True)
                ot = wpool.tile([128, EMB], F32, tag="ot", name="ot", bufs=3)
                nc.vector.tensor_add(ot[:], ph[:, :], bcast["bu"][:])
                nc.sync.dma_start(out_d[t0:t0 + 128, :], ot[:])

            for ti in range(NTI + 1):
                if ti < NTI:
                    emit_v1_s1(ti)
                if ti >= 1:
                    emit_av_out(ti - 1)

    return nc


_NC_CACHE = {}


def _get_program(with_bias):
    if with_bias not in _NC_CACHE:
        nc = build_bias() if with_bias else build_fast()
        nc.compile()          # bacc passes: wait splitting, reg alloc, ISA
        _NC_CACHE[with_bias] = nc
    return _NC_CACHE[with_bias]


def _make_masks():
    tri = np.triu(np.ones((KBLK, KBLK), np.float32))           # [c_l, r_l]
    m1 = np.kron(np.eye(2, dtype=np.float32), tri)             # [128, 128]
    # mask2[h][i, rl] = 1 if 64*i <= h*HALF + rl
    r = np.arange(HALF)
    m2 = []
    for h in range(2):
        blk = (h * HALF + r) // KBLK                           # [HALF]
        m2.append((np.arange(NSTART)[:, None] <= blk[None, :])
                  .astype(np.float32))
    return m1, m2


def make_in_maps(inputs, with_bias):
    x = np.asarray(inputs["x"], np.float32)
    wk = np.asarray(inputs["Wk"], np.float32)
    wq = np.asarray(inputs["Wq"], np.float32)
    wv = np.asarray(inputs["Wv"], np.float32)
    wu = np.asarray(inputs["Wu"], np.float32)
    bk = np.asarray(inputs["bk"], np.float32)
    bq = np.asarray(inputs["bq"], np.float32)
    bv = np.asarray(inputs["bv"], np.float32)
    bu = np.asarray(inputs["bu"], np.float32)

    # host-side weight preprocessing (pure weight-product folding)
    wvu1 = np.ascontiguousarray(wv.T @ wu[:, :EMB].T).astype(BF16NP)
    wvu2 = np.ascontiguousarray(wv.T @ wu[:, EMB:].T).astype(BF16NP)

    m1, m2 = _make_masks()
    starts = np.arange(NSTART) * KBLK

    in_maps = []
    for c in range(NCORES):
        b, h = c // 2, c % 2
        xin = np.concatenate(
            [x[b, h * HALF:(h + 1) * HALF], x[b, starts]], axis=0)
        m = {
            "xt": np.ascontiguousarray(xin.T.astype(BF16NP)),
            "wvu1": wvu1, "wvu2": wvu2,
            "mask1": m1.astype(BF16NP), "mask2": m2[h].astype(BF16NP),
        }
        if with_bias:
            m.update({
                "wkt": np.ascontiguousarray(wk.T).astype(BF16NP),
                "wqt": np.ascontiguousarray(wq.T).astype(BF16NP),
                "bkq": np.ascontiguousarray(np.concatenate(
                    [bk.reshape(EMB // 128, 128).T,
                     bq.reshape(EMB // 128, 128).T], axis=1)),
                "bv1r": (bv @ wu[:, :EMB].T).reshape(1, EMB).copy(),
                "bv2r": (bv @ wu[:, EMB:].T).reshape(1, EMB).copy(),
                "bur": bu.reshape(1, EMB).copy(),
                "ones": np.ones((1, 128), np.float32),
            })
        else:
            # At[f2, f1] = (Wk^T Wq)^T = Wq^T Wk
            m["at"] = np.ascontiguousarray(wq.T @ wk).astype(BF16NP)
        in_maps.append(m)
    return in_maps


def _ensure_ntff_hook():
    """The agent image lacks antenv.axon_hooks; synthesize it and register
    the ctypes NTFF profiling hook so trace=True works under axon."""
    import importlib.util
    if "antenv.axon_hooks" in sys.modules:
        return
    try:
        if importlib.util.find_spec("antenv.axon_hooks") is not None:
            return
    except ValueError:
        return
    import types
    import antenv
    m = types.ModuleType("antenv.axon_hooks")
    m._hook = None
    def set_axon_ntff_profile_hook(h):
        m._hook = h
    def get_axon_ntff_profile_hook():
        return m._hook
    m.set_axon_ntff_profile_hook = set_axon_ntff_profile_hook
    m.get_axon_ntff_profile_hook = get_axon_ntff_profile_hook
    sys.modules["antenv.axon_hooks"] = m
    antenv.axon_hooks = m
    try:
        from trn_agent_boot.trn_boot import _ntff_profile_via_ctypes
        m._hook = _ntff_profile_via_ctypes("/opt/axon/libaxon_pjrt.so")
    except Exception:
        pass


def run_sharded(inputs, trace=False, trace_kwargs=None):
    """inputs: dict of full numpy arrays keyed like setup_inputs().
    Returns (full_output [B, T, EMB] float32, BassKernelResults)."""
    if trace:
        _ensure_ntff_hook()
    with_bias = any(
        float(np.abs(np.asarray(inputs[k])).max()) != 0.0
        for k in ("bk", "bq", "bv", "bu"))
    in_maps = make_in_maps(inputs, with_bias)
    nc = _get_program(with_bias)
    res = run_bass_kernel_spmd(nc, in_maps, list(range(NCORES)), trace=trace,
                               **(trace_kwargs or {}))

    out = np.empty((B, T, EMB), np.float32)
    for c in range(NCORES):
        b, h = c // 2, c % 2
        out[b, h * HALF:(h + 1) * HALF] = res.results[c]["out"]
    return out, res


def kernel(**inputs):
    out, _ = run_sharded(inputs, trace=False)
    return out


# revision 30
# speedup vs baseline: 1.1437x; 1.0165x over previous
"""Trainium2 Bass kernel for nn_BlocksparseFixedSelfAttention.

Reference computation (B=4, T=2048, EMB=512, KBLK=64):
    Kt = x @ Wk.T + bk ; Qt = x @ Wq.T + bq ; Vt = x @ Wv.T + bv
    head1: block-causal local attention inside each 64-token block
           (row j attends cols [block_start(j) .. j], S = K Q^T)
    head2: row r attends every block start c = 64*i with c <= r
    out = concat(h1, h2) @ Wu.T + bu

Algebraic restructure (zero-bias fast path):
  1. Output projection folded into V. With Wu = [Wu1 | Wu2]:
         out = sum_blk tril(K_b Q_b^T) (V_b Wu1^T) + S2 (V_s Wu2^T) + bu
     so the device computes V1 = x @ (Wv^T Wu1^T) and V2s =
     x_starts @ (Wv^T Wu2^T); the two AV matmuls accumulate into one
     [128, 512] PSUM tile and there is no output GEMM at all.
  2. Score Gram trick: S = K Q^T = x (Wk^T Wq) x^T, so with
     A = Wk^T Wq precomputed on the host, a single projection
     y' = x @ A^T replaces BOTH the K and Q projections; scores are
     dots of y' against the raw (bf16) x already resident in SBUF:
         S1^T[c, r] = sum_f y't[f, c] xt[f, r]
         S2^T[i, r] = sum_f y't[f, start_i] xt[f, r]
  All matmul operands bf16 (host-converted), f32 PSUM accumulate.
  Measured: rel err ~4.6e-3 vs the f32 reference (tol 2e-2).

Per-core PE row budget: y' 16896 + V1 16384 + V2s 2048 + S2 4096 +
S1 4096 + AV 8192 = 51712 moving rows (~21.5 us at 2.4 GHz).

Sharding: data-parallel over (batch, T-half) -> 8 shards, one per core.
Each core gets its 1024 own token rows of x plus the 32 block-start
rows, feature-major (x^T), replicated (pre-folded) weights, and
produces its [1024, 512] slice of the output.

Nonzero biases (never hit by this problem's inputs, which have
fill=zeros biases) fall back to an explicit-K/Q program variant.
"""

import os
import sys

import numpy as np

for _p in ("/opt/trn_rl_repo",):
    if _p not in sys.path and os.path.isdir(_p):
        sys.path.append(_p)

import ml_dtypes

from concourse import bass, bacc, mybir
from concourse import tile
from concourse.bass_utils import run_bass_kernel_spmd

T = 2048
KBLK = 64
EMB = 512
B = 4
NCORES = 8
HALF = T // 2            # tokens owned per core
NSTART = T // KBLK       # 32 block starts
TOT = HALF + NSTART      # own tokens + appended block-start tokens
F32 = mybir.dt.float32
F32R = mybir.dt.float32r
BF16 = mybir.dt.bfloat16
BF16NP = ml_dtypes.bfloat16

NF = EMB // 128          # 4 feature chunks (contraction)
NE = EMB // 128          # 4 embed chunks
NTI = HALF // 128        # 8 own-token tiles


def build_fast():
    """Zero-bias A-form program."""
    nc = bacc.Bacc("TRN2", target_bir_lowering=False, debug=False)

    xt_d = nc.declare_dram_parameter("xt", [EMB, TOT], BF16, False)
    at_d = nc.declare_dram_parameter("at", [EMB, EMB], BF16, False)
    wvu1_d = nc.declare_dram_parameter("wvu1", [EMB, EMB], BF16, False)
    wvu2_d = nc.declare_dram_parameter("wvu2", [EMB, EMB], BF16, False)
    m1_d = nc.declare_dram_parameter("mask1", [128, 128], BF16, False)
    m2_d = nc.declare_dram_parameter("mask2", [NSTART, HALF], BF16, False)
    out_d = nc.declare_dram_parameter("out", [HALF, EMB], BF16, True)

    with tile.TileContext(nc) as tc:
        with (
            tc.tile_pool(name="const", bufs=1) as cpool,
            tc.tile_pool(name="big", bufs=1) as bpool,
            tc.tile_pool(name="work", bufs=3) as wpool,
            tc.tile_pool(name="ps", bufs=8, space="PSUM") as pspool,
        ):
            def psum(tag="ps"):
                return pspool.tile([128, 512], F32, tag=tag, name=tag, bufs=8)

            # ---- PE warmup: ramp the tensor engine's p-state while the
            # first DMAs are in flight (operands come from a memset, so
            # no DMA dependency). ----------------------------------------
            warm_sb = cpool.tile([128, 512], BF16, name="warm_sb")
            nc.vector.memset(warm_sb[:], 1.0)
            for _ in range(6):
                pw = psum()
                nc.tensor.matmul(pw[:, :512], warm_sb[:, :128],
                                 warm_sb[:, :512], start=True, stop=True)

            # ---- DMA: y'-phase operands (at, xt) first ------------------
            at_flat = cpool.tile([128, NF * EMB], BF16, name="at_flat")
            at_sb = [at_flat[:, fi * EMB:(fi + 1) * EMB] for fi in range(NF)]
            xt_flat = bpool.tile([128, NF * TOT], BF16, name="xt_flat")
            xt_sb = [xt_flat[:, fi * TOT:(fi + 1) * TOT] for fi in range(NF)]
            # the two queues each carry half of (at, xt) so the chunk
            # PAIRS complete in fi-consumption order instead of xt3
            # trailing the whole scalar stream
            nc.sync.dma_start(at_sb[0], at_d[0:128, :])
            nc.scalar.dma_start(xt_sb[0][:, :TOT], xt_d[0:128, :])
            nc.sync.dma_start(at_sb[1], at_d[128:256, :])
            nc.scalar.dma_start(xt_sb[1][:, :TOT], xt_d[128:256, :])
            nc.sync.dma_start(xt_sb[2][:, :TOT], xt_d[256:384, :])
            nc.scalar.dma_start(at_sb[2], at_d[256:384, :])
            nc.sync.dma_start(xt_sb[3][:, :TOT], xt_d[384:512, :])
            nc.scalar.dma_start(at_sb[3], at_d[384:512, :])

            def load_w(name, dram, eng):
                t_ = cpool.tile([128, NF * EMB], BF16, name=name)
                chunks = [t_[:, ci * EMB:(ci + 1) * EMB] for ci in range(NF)]
                for ci in range(NF):
                    eng.dma_start(chunks[ci], dram[ci * 128:(ci + 1) * 128, :])
                return chunks

            wvu2_sb = load_w("wvu2_sb", wvu2_d, nc.scalar)
            m2_sb = cpool.tile([NSTART, HALF], BF16, name="m2_sb")
            nc.sync.dma_start(m2_sb[:], m2_d[:])
            m1_sb = cpool.tile([128, 128], BF16, name="m1_sb")
            nc.sync.dma_start(m1_sb[:], m1_d[:])
            wvu1_sb = load_w("wvu1_sb", wvu1_d, nc.sync)

            # ---- zero-pad tiles: s2t and v2s are padded to 128
            # partitions (rows 32.. stay zero) so the head2 AV matmuls use
            # the same 128x128 PE tile config as everything else — a
            # 32-row stationary forces a tile-size switch that blocks
            # LDWEIGHTS prefetch and costs ~100 ns per matmul ------------
            v2s_sb = cpool.tile([128, EMB], BF16, name="v2s_sb")
            nc.vector.memset(v2s_sb[:], 0.0)
            s2t_sb = bpool.tile([128, HALF], BF16, name="s2t_sb")
            nc.vector.memset(s2t_sb[:], 0.0)

            # ---- y' projection: y't[f1, c] = sum_f2 At[f2, f1] xt[f2, c]
            # (ei, fi) outer with the three t-spans inside: consecutive
            # matmuls share one stationary, so the 32-wide tail's
            # LDWEIGHTS hides under the preceding 512-row matmul. ---------
            yt_sb = [bpool.tile([128, TOT], BF16, name=f"yt_sb{ei}")
                     for ei in range(NE)]
            spans = [(0, 512), (512, 512), (1024, NSTART)]
            for ei in range(NE):
                pss = [psum() for _ in spans]
                for fi in range(NF):
                    for si, (t0, sw) in enumerate(spans):
                        nc.tensor.matmul(
                            pss[si][:, :sw],
                            at_sb[fi][:, ei * 128:(ei + 1) * 128],
                            xt_sb[fi][:, t0:t0 + sw],
                            start=(fi == 0), stop=(fi == NF - 1))
                for si, (t0, sw) in enumerate(spans):
                    nc.vector.tensor_copy(yt_sb[ei][:, t0:t0 + sw],
                                          pss[si][:, :sw])

            # ---- V2s = x_starts @ (Wv^T Wu2^T); stationary is the padded
            # [128]-wide starts block so the PE tile config stays 128x128 -
            ps = psum()
            for fi in range(NF):
                nc.tensor.matmul(ps[:NSTART, :], xt_sb[fi][:, HALF:TOT],
                                 wvu2_sb[fi][:],
                                 start=(fi == 0), stop=(fi == NF - 1))
            nc.scalar.copy(v2s_sb[:NSTART, :], ps[:NSTART, :])

            # ---- head2 scores S2^T[i, r] = y'_starts . x_r, masked ------
            for t0 in (0, 512):
                ps2 = psum()
                for fi in range(NF):
                    nc.tensor.matmul(ps2[:NSTART, :512],
                                     yt_sb[fi][:, HALF:TOT],
                                     xt_sb[fi][:, t0:t0 + 512],
                                     start=(fi == 0), stop=(fi == NF - 1))
                nc.vector.tensor_mul(s2t_sb[:NSTART, t0:t0 + 512],
                                     ps2[:NSTART, :512],
                                     m2_sb[:, t0:t0 + 512])

            # ---- main tile pipeline. Per iteration the PE stream is
            #   V1fi0 s1fi0 V1fi1 [h2AV'] s1fi1 V1fi2 [h1AV'] s1fi2
            #   V1fi3 s1fi3                       (' = tile ti-1)
            # Each short-stream matmul (128-row score, AV) directly
            # follows a 512-row V1 matmul so its LDWEIGHTS prefetches
            # under the long stream and the weight-load port never
            # saturates. The two AV matmuls accumulate into one PSUM
            # bank; scalar drains it to bf16 and sync stores it. ---------
            v1n_sb = [bpool.tile([128, EMB], BF16, name=f"v1n_sb{ti}")
                      for ti in range(NTI)]
            s1ts = [None] * NTI

            def emit_v1_s1(ti):
                t0 = ti * 128
                ps = psum()
                for fi in range(NF):
                    nc.tensor.matmul(ps[:, :], xt_sb[fi][:, t0:t0 + 128],
                                     wvu1_sb[fi][:],
                                     start=(fi == 0), stop=(fi == NF - 1))
                nc.scalar.copy(v1n_sb[ti][:], ps[:, :])
                ps1 = psum()
                for fi in range(NF):
                    nc.tensor.matmul(ps1[:, :128],
                                     yt_sb[fi][:, t0:t0 + 128],
                                     xt_sb[fi][:, t0:t0 + 128],
                                     start=(fi == 0), stop=(fi == NF - 1))
                s1t = wpool.tile([128, 128], BF16, tag="s1t", name="s1t",
                                 bufs=4)
                nc.vector.tensor_mul(s1t[:], ps1[:, :128], m1_sb[:])
                s1ts[ti] = s1t

            def emit_av_out(ti):
                t0 = ti * 128
                ph = psum()
                ot = wpool.tile([128, EMB], BF16, tag="ot", name="ot", bufs=3)
                nc.tensor.matmul(ph[:, :], s1ts[ti][:], v1n_sb[ti][:],
                                 start=True, stop=False)
                nc.tensor.matmul(ph[:, :], s2t_sb[:, t0:t0 + 128],
                                 v2s_sb[:], start=False, stop=True)
                if ti < NTI - 1:
                    nc.scalar.copy(ot[:], ph[:, :])
                    nc.sync.dma_start(out_d[t0:t0 + 128, :], ot[:])
                else:
                    # last tile: split the drain across two engines and two
                    # DMA queues to shorten the serial tail
                    nc.vector.tensor_copy(ot[:, :256], ph[:, :256])
                    nc.sync.dma_start(out_d[t0:t0 + 128, :256], ot[:, :256])
                    nc.scalar.copy(ot[:, 256:], ph[:, 256:])
                    nc.scalar.dma_start(out_d[t0:t0 + 128, 256:],
                                        ot[:, 256:])

            for ti in range(NTI + 1):
                if ti < NTI:
                    emit_v1_s1(ti)
                if ti >= 1:
                    emit_av_out(ti - 1)

    return nc


def build_bias():
    """Generic program with explicit K/Q projections and bias adds
    (correctness fallback; the harness's biases are all zero)."""
    nc = bacc.Bacc("TRN2", target_bir_lowering=False, debug=False)

    xt_d = nc.declare_dram_parameter("xt", [EMB, TOT], BF16, False)
    wkt_d = nc.declare_dram_parameter("wkt", [EMB, EMB], BF16, False)
    wqt_d = nc.declare_dram_parameter("wqt", [EMB, EMB], BF16, False)
    wvu1_d = nc.declare_dram_parameter("wvu1", [EMB, EMB], BF16, False)
    wvu2_d = nc.declare_dram_parameter("wvu2", [EMB, EMB], BF16, False)
    m1_d = nc.declare_dram_parameter("mask1", [128, 128], BF16, False)
    m2_d = nc.declare_dram_parameter("mask2", [NSTART, HALF], BF16, False)
    bkq_d = nc.declare_dram_parameter("bkq", [128, 2 * NE], F32, False)
    bv1_d = nc.declare_dram_parameter("bv1r", [1, EMB], F32, False)
    bv2_d = nc.declare_dram_parameter("bv2r", [1, EMB], F32, False)
    bu_d = nc.declare_dram_parameter("bur", [1, EMB], F32, False)
    ones_d = nc.declare_dram_parameter("ones", [1, 128], F32, False)
    out_d = nc.declare_dram_parameter("out", [HALF, EMB], F32, True)

    with tile.TileContext(nc) as tc:
        with (
            tc.tile_pool(name="const", bufs=1) as cpool,
            tc.tile_pool(name="big", bufs=1) as bpool,
            tc.tile_pool(name="work", bufs=3) as wpool,
            tc.tile_pool(name="ps", bufs=8, space="PSUM") as pspool,
        ):
            def psum(tag="ps"):
                return pspool.tile([128, 512], F32, tag=tag, name=tag, bufs=8)

            wkt_flat = cpool.tile([128, NF * EMB], BF16, name="wkt_flat")
            wkt_sb = [wkt_flat[:, fi * EMB:(fi + 1) * EMB] for fi in range(NF)]
            xt_flat = bpool.tile([128, NF * TOT], BF16, name="xt_flat")
            xt_sb = [xt_flat[:, fi * TOT:(fi + 1) * TOT] for fi in range(NF)]
            for fi in range(NF):
                nc.sync.dma_start(wkt_sb[fi], wkt_d[fi * 128:(fi + 1) * 128, :])
                nc.scalar.dma_start(xt_sb[fi], xt_d[fi * 128:(fi + 1) * 128, :])

            def load_w(name, dram, eng):
                t_ = cpool.tile([128, NF * EMB], BF16, name=name)
                chunks = [t_[:, ci * EMB:(ci + 1) * EMB] for ci in range(NF)]
                for ci in range(NF):
                    eng.dma_start(chunks[ci], dram[ci * 128:(ci + 1) * 128, :])
                return chunks

            wqt_sb = load_w("wqt_sb", wqt_d, nc.sync)
            bkq_sb = cpool.tile([128, 2 * NE], F32, name="bkq_sb")
            nc.sync.dma_start(bkq_sb[:], bkq_d[:])
            bkc_sb = bkq_sb[:, 0:NE]
            bqc_sb = bkq_sb[:, NE:2 * NE]
            m2_sb = cpool.tile([NSTART, HALF], BF16, name="m2_sb")
            nc.sync.dma_start(m2_sb[:], m2_d[:])
            m1_sb = cpool.tile([128, 128], BF16, name="m1_sb")
            nc.sync.dma_start(m1_sb[:], m1_d[:])
            ones_sb = cpool.tile([1, 128], F32R, name="ones_sb")
            nc.sync.dma_start(ones_sb[:], ones_d[:].bitcast(F32R))
            bv1r_sb = cpool.tile([1, EMB], F32R, name="bv1r_sb")
            nc.sync.dma_start(bv1r_sb[:], bv1_d[:].bitcast(F32R))
            bv2r_sb = cpool.tile([1, EMB], F32R, name="bv2r_sb")
            nc.sync.dma_start(bv2r_sb[:], bv2_d[:].bitcast(F32R))
            bur_sb = cpool.tile([1, EMB], F32R, name="bur_sb")
            nc.sync.dma_start(bur_sb[:], bu_d[:].bitcast(F32R))
            wvu1_sb = load_w("wvu1_sb", wvu1_d, nc.scalar)
            wvu2_sb = load_w("wvu2_sb", wvu2_d, nc.scalar)

            kt_sb = [bpool.tile([128, HALF], BF16, name=f"kt_sb{ei}")
                     for ei in range(NE)]
            for t0 in (0, 512):
                pss = [psum() for _ in range(NE)]
                for fi in range(NF):
                    for ei in range(NE):
                        nc.tensor.matmul(
                            pss[ei][:, :512],
                            wkt_sb[fi][:, ei * 128:(ei + 1) * 128],
                            xt_sb[fi][:, t0:t0 + 512],
                            start=(fi == 0), stop=(fi == NF - 1))
                for ei in range(NE):
                    nc.vector.tensor_scalar_add(
                        kt_sb[ei][:, t0:t0 + 512], pss[ei][:, :512],
                        bkc_sb[:, ei:ei + 1])

            bcast = {}
            for nm, src in (("bv1", bv1r_sb), ("bv2", bv2r_sb),
                            ("bu", bur_sb)):
                pb = psum()
                nc.tensor.matmul(pb[:, :EMB], ones_sb[:1, :], src[:1, :],
                                 start=True, stop=True)
                bb = cpool.tile([128, EMB], F32, name=f"{nm}b_sb")
                nc.vector.tensor_copy(bb[:], pb[:, :EMB])
                bcast[nm] = bb

            qt_sb = [bpool.tile([128, TOT], BF16, name=f"qt_sb{ei}")
                     for ei in range(NE)]
            spans = [(0, 512), (512, 512), (1024, NSTART)]
            for ei in range(NE):
                pss = [psum() for _ in spans]
                for fi in range(NF):
                    for si, (t0, sw) in enumerate(spans):
                        nc.tensor.matmul(
                            pss[si][:, :sw],
                            wqt_sb[fi][:, ei * 128:(ei + 1) * 128],
                            xt_sb[fi][:, t0:t0 + sw],
                            start=(fi == 0), stop=(fi == NF - 1))
                for si, (t0, sw) in enumerate(spans):
                    nc.vector.tensor_scalar_add(
                        qt_sb[ei][:, t0:t0 + sw], pss[si][:, :sw],
                        bqc_sb[:, ei:ei + 1])

            ps = psum()
            for fi in range(NF):
                nc.tensor.matmul(ps[:NSTART, :], xt_sb[fi][:, HALF:TOT],
                                 wvu2_sb[fi][:],
                                 start=(fi == 0), stop=(fi == NF - 1))
            v2s_sb = cpool.tile([NSTART, EMB], BF16, name="v2s_sb")
            nc.vector.tensor_add(v2s_sb[:], ps[:NSTART, :],
                                 bcast["bv2"][:NSTART, :])

            s2t_sb = bpool.tile([NSTART, HALF], BF16, name="s2t_sb")
            for t0 in (0, 512):
                ps2 = psum()
                for ei in range(NE):
                    nc.tensor.matmul(ps2[:NSTART, :512],
                                     qt_sb[ei][:, HALF:TOT],
                                     kt_sb[ei][:, t0:t0 + 512],
                                     start=(ei == 0), stop=(ei == NE - 1))
                nc.vector.tensor_mul(s2t_sb[:, t0:t0 + 512], ps2[:NSTART, :512],
                                     m2_sb[:, t0:t0 + 512])

            v1n_sb = [bpool.tile([128, EMB], BF16, name=f"v1n_sb{ti}")
                      for ti in range(NTI)]
            s1ts = [None] * NTI

            def emit_v1_s1(ti):
                t0 = ti * 128
                ps = psum()
                for fi in range(NF):
                    nc.tensor.matmul(ps[:, :], xt_sb[fi][:, t0:t0 + 128],
                                     wvu1_sb[fi][:],
                                     start=(fi == 0), stop=(fi == NF - 1))
                nc.vector.tensor_add(v1n_sb[ti][:], ps[:, :], bcast["bv1"][:])
                ps1 = psum()
                for ei in range(NE):
                    nc.tensor.matmul(ps1[:, :128],
                                     qt_sb[ei][:, t0:t0 + 128],
                                     kt_sb[ei][:, t0:t0 + 128],
                                     start=(ei == 0), stop=(ei == NE - 1))
                s1t = wpool.tile([128, 128], BF16, tag="s1t", name="s1t",
                                 bufs=4)
                nc.vector.tensor_mul(s1t[:], ps1[:, :128], m1_sb[:])
                s1ts[ti] = s1t

            def emit_av_out(ti):
                t0 = ti * 128
                ph = psum()
                nc.tensor.matmul(ph[:, :], s1ts[ti][:], v1n_sb[ti][:],
                                 start=True, stop=False)
                nc.tensor.matmul(ph[:, :], s2t_sb[:, t0:t0 + 128],
                                 v2s_sb[:], start=False, stop=True)
                ot = wpool.tile([128, EMB], F32, tag="ot", name="ot", bufs=3)
                nc.vector.tensor_add(ot[:], ph[:, :], bcast["bu"][:])
                nc.sync.dma_start(out_d[t0:t0 + 128, :], ot[:])

            for ti in range(NTI + 1):
                if ti < NTI:
                    emit_v1_s1(ti)
                if ti >= 1:
                    emit_av_out(ti - 1)

    return nc


_NC_CACHE = {}


def _get_program(with_bias):
    if with_bias not in _NC_CACHE:
        nc = build_bias() if with_bias else build_fast()
        nc.compile()          # bacc passes: wait splitting, reg alloc, ISA
        _NC_CACHE[with_bias] = nc
    return _NC_CACHE[with_bias]


def _make_masks():
    tri = np.triu(np.ones((KBLK, KBLK), np.float32))           # [c_l, r_l]
    m1 = np.kron(np.eye(2, dtype=np.float32), tri)             # [128, 128]
    # mask2[h][i, rl] = 1 if 64*i <= h*HALF + rl
    r = np.arange(HALF)
    m2 = []
    for h in range(2):
        blk = (h * HALF + r) // KBLK                           # [HALF]
        m2.append((np.arange(NSTART)[:, None] <= blk[None, :])
                  .astype(np.float32))
    return m1, m2


def make_in_maps(inputs, with_bias):
    x = np.asarray(inputs["x"], np.float32)
    wk = np.asarray(inputs["Wk"], np.float32)
    wq = np.asarray(inputs["Wq"], np.float32)
    wv = np.asarray(inputs["Wv"], np.float32)
    wu = np.asarray(inputs["Wu"], np.float32)
    bk = np.asarray(inputs["bk"], np.float32)
    bq = np.asarray(inputs["bq"], np.float32)
    bv = np.asarray(inputs["bv"], np.float32)
    bu = np.asarray(inputs["bu"], np.float32)

    # host-side weight preprocessing (pure weight-product folding)
    wvu1 = np.ascontiguousarray(wv.T @ wu[:, :EMB].T).astype(BF16NP)
    wvu2 = np.ascontiguousarray(wv.T @ wu[:, EMB:].T).astype(BF16NP)

    m1, m2 = _make_masks()
    starts = np.arange(NSTART) * KBLK

    in_maps = []
    for c in range(NCORES):
        b, h = c // 2, c % 2
        xin = np.concatenate(
            [x[b, h * HALF:(h + 1) * HALF], x[b, starts]], axis=0)
        m = {
            "xt": np.ascontiguousarray(xin.T.astype(BF16NP)),
            "wvu1": wvu1, "wvu2": wvu2,
            "mask1": m1.astype(BF16NP), "mask2": m2[h].astype(BF16NP),
        }
        if with_bias:
            m.update({
                "wkt": np.ascontiguousarray(wk.T).astype(BF16NP),
                "wqt": np.ascontiguousarray(wq.T).astype(BF16NP),
                "bkq": np.ascontiguousarray(np.concatenate(
                    [bk.reshape(EMB // 128, 128).T,
                     bq.reshape(EMB // 128, 128).T], axis=1)),
                "bv1r": (bv @ wu[:, :EMB].T).reshape(1, EMB).copy(),
                "bv2r": (bv @ wu[:, EMB:].T).reshape(1, EMB).copy(),
                "bur": bu.reshape(1, EMB).copy(),
                "ones": np.ones((1, 128), np.float32),
            })
        else:
            # At[f2, f1] = (Wk^T Wq)^T = Wq^T Wk
            m["at"] = np.ascontiguousarray(wq.T @ wk).astype(BF16NP)
        in_maps.append(m)
    return in_maps


def _ensure_ntff_hook():
    """The agent image lacks antenv.axon_hooks; synthesize it and register
    the ctypes NTFF profiling hook so trace=True works under axon."""
    import importlib.util
    if "antenv.axon_hooks" in sys.modules:
        return
    try:
        if importlib.util.find_spec("antenv.axon_hooks") is not None:
            return
    except ValueError:
        return
    import types
    import antenv
    m = types.ModuleType("antenv.axon_hooks")
    m._hook = None
    def set_axon_ntff_profile_hook(h):
        m._hook = h
    def get_axon_ntff_profile_hook():
        return m._hook
    m.set_axon_ntff_profile_hook = set_axon_ntff_profile_hook
    m.get_axon_ntff_profile_hook = get_axon_ntff_profile_hook
    sys.modules["antenv.axon_hooks"] = m
    antenv.axon_hooks = m
    try:
        from trn_agent_boot.trn_boot import _ntff_profile_via_ctypes
        m._hook = _ntff_profile_via_ctypes("/opt/axon/libaxon_pjrt.so")
    except Exception:
        pass


def run_sharded(inputs, trace=False, trace_kwargs=None):
    """inputs: dict of full numpy arrays keyed like setup_inputs().
    Returns (full_output [B, T, EMB] float32, BassKernelResults)."""
    if trace:
        _ensure_ntff_hook()
    with_bias = any(
        float(np.abs(np.asarray(inputs[k])).max()) != 0.0
        for k in ("bk", "bq", "bv", "bu"))
    in_maps = make_in_maps(inputs, with_bias)
    nc = _get_program(with_bias)
    res = run_bass_kernel_spmd(nc, in_maps, list(range(NCORES)), trace=trace,
                               **(trace_kwargs or {}))

    out = np.empty((B, T, EMB), np.float32)
    for c in range(NCORES):
        b, h = c // 2, c % 2
        out[b, h * HALF:(h + 1) * HALF] = res.results[c]["out"]
    return out, res


def kernel(**inputs):
    out, _ = run_sharded(inputs, trace=False)
    return out


# revision 36
# speedup vs baseline: 1.2561x; 1.0983x over previous
"""Trainium2 Bass kernel for nn_BlocksparseFixedSelfAttention.

Reference computation (B=4, T=2048, EMB=512, KBLK=64):
    Kt = x @ Wk.T + bk ; Qt = x @ Wq.T + bq ; Vt = x @ Wv.T + bv
    head1: block-causal local attention inside each 64-token block
           (row j attends cols [block_start(j) .. j], S = K Q^T)
    head2: row r attends every block start c = 64*i with c <= r
    out = concat(h1, h2) @ Wu.T + bu

Algebraic restructure (zero-bias fast path):
  1. Output projection folded into V. With Wu = [Wu1 | Wu2]:
         out = sum_blk tril(K_b Q_b^T) (V_b Wu1^T) + S2 (V_s Wu2^T) + bu
     so the device computes V1 = x @ (Wv^T Wu1^T) and V2s =
     x_starts @ (Wv^T Wu2^T); the two AV matmuls accumulate into one
     [128, 512] PSUM tile and there is no output GEMM at all.
  2. Score Gram trick: S = K Q^T = x (Wk^T Wq) x^T, so with
     A = Wk^T Wq precomputed on the host, a single projection
     y' = x @ A^T replaces BOTH the K and Q projections; scores are
     dots of y' against the raw (bf16) x already resident in SBUF:
         S1^T[c, r] = sum_f y't[f, c] xt[f, r]
         S2^T[i, r] = sum_f y't[f, start_i] xt[f, r]
  All matmul operands bf16 (host-converted), f32 PSUM accumulate.
  Measured: rel err ~4.6e-3 vs the f32 reference (tol 2e-2).

Per-core PE row budget: y' 16896 + V1 16384 + V2s 2048 + S2 4096 +
S1 4096 + AV 8192 = 51712 moving rows (~21.5 us at 2.4 GHz).

Sharding: data-parallel over (batch, T-half) -> 8 shards, one per core.
Each core gets its 1024 own token rows of x plus the 32 block-start
rows, feature-major (x^T), replicated (pre-folded) weights, and
produces its [1024, 512] slice of the output.

Nonzero biases (never hit by this problem's inputs, which have
fill=zeros biases) fall back to an explicit-K/Q program variant.
"""

import os
import sys

import numpy as np

for _p in ("/opt/trn_rl_repo",):
    if _p not in sys.path and os.path.isdir(_p):
        sys.path.append(_p)

import ml_dtypes

from concourse import bass, bacc, mybir
from concourse import tile
from concourse.bass_utils import run_bass_kernel_spmd

T = 2048
KBLK = 64
EMB = 512
B = 4
NCORES = 8
HALF = T // 2            # tokens owned per core
NSTART = T // KBLK       # 32 block starts
TOT = HALF + NSTART      # own tokens + appended block-start tokens
F32 = mybir.dt.float32
F32R = mybir.dt.float32r
BF16 = mybir.dt.bfloat16
BF16NP = ml_dtypes.bfloat16

NF = EMB // 128          # 4 feature chunks (contraction)
NE = EMB // 128          # 4 embed chunks
NTI = HALF // 128        # 8 own-token tiles


def build_fast():
    """Zero-bias A-form program."""
    nc = bacc.Bacc("TRN2", target_bir_lowering=False, debug=False)

    xt_d = nc.declare_dram_parameter("xt", [EMB, TOT], BF16, False)
    at_d = nc.declare_dram_parameter("at", [EMB, EMB], BF16, False)
    wvu1_d = nc.declare_dram_parameter("wvu1", [EMB, EMB], BF16, False)
    wvu2_d = nc.declare_dram_parameter("wvu2", [EMB, EMB], BF16, False)
    m1_d = nc.declare_dram_parameter("mask1", [128, 128], BF16, False)
    m2_d = nc.declare_dram_parameter("mask2", [NSTART, HALF], BF16, False)
    out_d = nc.declare_dram_parameter("out", [HALF, EMB], BF16, True)

    with tile.TileContext(nc) as tc:
        with (
            tc.tile_pool(name="const", bufs=1) as cpool,
            tc.tile_pool(name="big", bufs=1) as bpool,
            tc.tile_pool(name="work", bufs=3) as wpool,
            tc.tile_pool(name="ps", bufs=8, space="PSUM") as pspool,
        ):
            def psum(tag="ps"):
                return pspool.tile([128, 512], F32, tag=tag, name=tag, bufs=8)

            # ---- PE warmup: ramp the tensor engine's p-state while the
            # first DMAs are in flight (operands come from a memset, so
            # no DMA dependency). ----------------------------------------
            warm_sb = cpool.tile([128, 512], BF16, name="warm_sb")
            nc.vector.memset(warm_sb[:], 1.0)
            for _ in range(7):
                pw = psum()
                nc.tensor.matmul(pw[:, :512], warm_sb[:, :128],
                                 warm_sb[:, :512], start=True, stop=True)

            # ---- DMA: y'-phase operands (at, xt) first ------------------
            TOTP = TOT + (128 - NSTART)          # 1152: padded starts block
            at_flat = cpool.tile([128, NF * EMB], BF16, name="at_flat")
            at_sb = [at_flat[:, fi * EMB:(fi + 1) * EMB] for fi in range(NF)]
            xt_flat = bpool.tile([128, NF * TOTP], BF16, name="xt_flat")
            xt_sb = [xt_flat[:, fi * TOTP:(fi + 1) * TOTP] for fi in range(NF)]
            for fi in range(NF):
                nc.vector.memset(xt_sb[fi][:, TOT:TOTP], 0.0)
            # the two queues each carry half of (at, xt) so the chunk
            # PAIRS complete in fi-consumption order instead of xt3
            # trailing the whole scalar stream
            nc.sync.dma_start(at_sb[0], at_d[0:128, :])
            nc.scalar.dma_start(xt_sb[0][:, :TOT], xt_d[0:128, :])
            nc.sync.dma_start(at_sb[1], at_d[128:256, :])
            nc.scalar.dma_start(xt_sb[1][:, :TOT], xt_d[128:256, :])
            nc.sync.dma_start(xt_sb[2][:, :TOT], xt_d[256:384, :])
            nc.scalar.dma_start(at_sb[2], at_d[256:384, :])
            nc.sync.dma_start(xt_sb[3][:, :TOT], xt_d[384:512, :])
            nc.scalar.dma_start(at_sb[3], at_d[384:512, :])

            def load_w(name, dram, eng):
                t_ = cpool.tile([128, NF * EMB], BF16, name=name)
                chunks = [t_[:, ci * EMB:(ci + 1) * EMB] for ci in range(NF)]
                for ci in range(NF):
                    eng.dma_start(chunks[ci], dram[ci * 128:(ci + 1) * 128, :])
                return chunks

            wvu2_sb = load_w("wvu2_sb", wvu2_d, nc.scalar)
            m2_sb = cpool.tile([NSTART, HALF], BF16, name="m2_sb")
            nc.sync.dma_start(m2_sb[:], m2_d[:])
            m1_sb = cpool.tile([128, 128], BF16, name="m1_sb")
            nc.sync.dma_start(m1_sb[:], m1_d[:])
            wvu1_sb = load_w("wvu1_sb", wvu1_d, nc.sync)

            # ---- zero-pad tiles: s2t and v2s are padded to 128
            # partitions (rows 32.. stay zero) so the head2 AV matmuls use
            # the same 128x128 PE tile config as everything else — a
            # 32-row stationary forces a tile-size switch that blocks
            # LDWEIGHTS prefetch and costs ~100 ns per matmul ------------
            v2s_sb = cpool.tile([128, EMB], BF16, name="v2s_sb")
            nc.vector.memset(v2s_sb[:], 0.0)
            s2t_sb = bpool.tile([128, HALF], BF16, name="s2t_sb")
            nc.vector.memset(s2t_sb[:], 0.0)

            # ---- y' projection: y't[f1, c] = sum_f2 At[f2, f1] xt[f2, c]
            # (ei, fi) outer with the three t-spans inside: consecutive
            # matmuls share one stationary, so the 32-wide tail's
            # LDWEIGHTS hides under the preceding 512-row matmul. ---------
            yt_sb = [bpool.tile([128, TOTP], BF16, name=f"yt_sb{ei}")
                     for ei in range(NE)]
            for ei in range(NE):
                nc.vector.memset(yt_sb[ei][:, TOT:TOTP], 0.0)
            spans = [(0, 512), (512, 512), (1024, NSTART)]
            for eig in range(NE // 2):
                eis = (2 * eig, 2 * eig + 1)
                pss = {(ei, si): psum() for ei in eis for si in range(3)}
                for fi in range(NF):
                    for ei in eis:
                        for si, (t0, sw) in enumerate(spans):
                            nc.tensor.matmul(
                                pss[ei, si][:, :sw],
                                at_sb[fi][:, ei * 128:(ei + 1) * 128],
                                xt_sb[fi][:, t0:t0 + sw],
                                start=(fi == 0), stop=(fi == NF - 1))
                for ei in eis:
                    eng = nc.vector if ei % 2 == 0 else nc.scalar
                    for si, (t0, sw) in enumerate(spans):
                        if eng is nc.vector:
                            nc.vector.tensor_copy(yt_sb[ei][:, t0:t0 + sw],
                                                  pss[ei, si][:, :sw])
                        else:
                            nc.scalar.copy(yt_sb[ei][:, t0:t0 + sw],
                                           pss[ei, si][:, :sw])

            # ---- V2s = x_starts @ (Wv^T Wu2^T); stationary is the padded
            # [128]-wide starts block so the PE tile config stays 128x128 -
            ps = psum()
            for fi in range(NF):
                nc.tensor.matmul(ps[:, :], xt_sb[fi][:, HALF:TOTP],
                                 wvu2_sb[fi][:],
                                 start=(fi == 0), stop=(fi == NF - 1))
            nc.scalar.copy(v2s_sb[:NSTART, :], ps[:NSTART, :])

            # ---- head2 scores S2^T[i, r] = y'_starts . x_r, masked ------
            for t0 in (0, 512):
                ps2 = psum()
                for fi in range(NF):
                    nc.tensor.matmul(ps2[:, :512],
                                     yt_sb[fi][:, HALF:TOTP],
                                     xt_sb[fi][:, t0:t0 + 512],
                                     start=(fi == 0), stop=(fi == NF - 1))
                nc.vector.tensor_mul(s2t_sb[:NSTART, t0:t0 + 512],
                                     ps2[:NSTART, :512],
                                     m2_sb[:, t0:t0 + 512])

            # ---- main tile pipeline. Per iteration the PE stream is
            #   V1fi0 s1fi0 V1fi1 [h2AV'] s1fi1 V1fi2 [h1AV'] s1fi2
            #   V1fi3 s1fi3                       (' = tile ti-1)
            # Each short-stream matmul (128-row score, AV) directly
            # follows a 512-row V1 matmul so its LDWEIGHTS prefetches
            # under the long stream and the weight-load port never
            # saturates. The two AV matmuls accumulate into one PSUM
            # bank; scalar drains it to bf16 and sync stores it. ---------
            v1n_sb = [bpool.tile([128, EMB], BF16, name=f"v1n_sb{ti}")
                      for ti in range(NTI)]
            s1ts = [None] * NTI

            def emit_v1_s1(ti):
                t0 = ti * 128
                ps = psum()
                for fi in range(NF):
                    nc.tensor.matmul(ps[:, :], xt_sb[fi][:, t0:t0 + 128],
                                     wvu1_sb[fi][:],
                                     start=(fi == 0), stop=(fi == NF - 1))
                nc.scalar.copy(v1n_sb[ti][:], ps[:, :])
                ps1 = psum()
                for fi in range(NF):
                    nc.tensor.matmul(ps1[:, :128],
                                     yt_sb[fi][:, t0:t0 + 128],
                                     xt_sb[fi][:, t0:t0 + 128],
                                     start=(fi == 0), stop=(fi == NF - 1))
                s1t = wpool.tile([128, 128], BF16, tag="s1t", name="s1t",
                                 bufs=4)
                nc.vector.tensor_mul(s1t[:], ps1[:, :128], m1_sb[:])
                s1ts[ti] = s1t

            def emit_av_out(ti):
                t0 = ti * 128
                ph = psum()
                ot = wpool.tile([128, EMB], BF16, tag="ot", name="ot", bufs=3)
                nc.tensor.matmul(ph[:, :], s1ts[ti][:], v1n_sb[ti][:],
                                 start=True, stop=False)
                nc.tensor.matmul(ph[:, :], s2t_sb[:, t0:t0 + 128],
                                 v2s_sb[:], start=False, stop=True)
                if ti < NTI - 1:
                    nc.vector.tensor_copy(ot[:], ph[:, :])
                    nc.sync.dma_start(out_d[t0:t0 + 128, :], ot[:])
                else:
                    # last tile: split the drain across two engines and two
                    # DMA queues to shorten the serial tail
                    nc.vector.tensor_copy(ot[:, :256], ph[:, :256])
                    nc.sync.dma_start(out_d[t0:t0 + 128, :256], ot[:, :256])
                    nc.scalar.copy(ot[:, 256:], ph[:, 256:])
                    nc.scalar.dma_start(out_d[t0:t0 + 128, 256:],
                                        ot[:, 256:])

            for ti in range(NTI + 1):
                if ti < NTI:
                    emit_v1_s1(ti)
                if ti >= 1:
                    emit_av_out(ti - 1)

    return nc


def build_bias():
    """Generic program with explicit K/Q projections and bias adds
    (correctness fallback; the harness's biases are all zero)."""
    nc = bacc.Bacc("TRN2", target_bir_lowering=False, debug=False)

    xt_d = nc.declare_dram_parameter("xt", [EMB, TOT], BF16, False)
    wkt_d = nc.declare_dram_parameter("wkt", [EMB, EMB], BF16, False)
    wqt_d = nc.declare_dram_parameter("wqt", [EMB, EMB], BF16, False)
    wvu1_d = nc.declare_dram_parameter("wvu1", [EMB, EMB], BF16, False)
    wvu2_d = nc.declare_dram_parameter("wvu2", [EMB, EMB], BF16, False)
    m1_d = nc.declare_dram_parameter("mask1", [128, 128], BF16, False)
    m2_d = nc.declare_dram_parameter("mask2", [NSTART, HALF], BF16, False)
    bkq_d = nc.declare_dram_parameter("bkq", [128, 2 * NE], F32, False)
    bv1_d = nc.declare_dram_parameter("bv1r", [1, EMB], F32, False)
    bv2_d = nc.declare_dram_parameter("bv2r", [1, EMB], F32, False)
    bu_d = nc.declare_dram_parameter("bur", [1, EMB], F32, False)
    ones_d = nc.declare_dram_parameter("ones", [1, 128], F32, False)
    out_d = nc.declare_dram_parameter("out", [HALF, EMB], F32, True)

    with tile.TileContext(nc) as tc:
        with (
            tc.tile_pool(name="const", bufs=1) as cpool,
            tc.tile_pool(name="big", bufs=1) as bpool,
            tc.tile_pool(name="work", bufs=3) as wpool,
            tc.tile_pool(name="ps", bufs=8, space="PSUM") as pspool,
        ):
            def psum(tag="ps"):
                return pspool.tile([128, 512], F32, tag=tag, name=tag, bufs=8)

            wkt_flat = cpool.tile([128, NF * EMB], BF16, name="wkt_flat")
            wkt_sb = [wkt_flat[:, fi * EMB:(fi + 1) * EMB] for fi in range(NF)]
            xt_flat = bpool.tile([128, NF * TOT], BF16, name="xt_flat")
            xt_sb = [xt_flat[:, fi * TOT:(fi + 1) * TOT] for fi in range(NF)]
            for fi in range(NF):
                nc.sync.dma_start(wkt_sb[fi], wkt_d[fi * 128:(fi + 1) * 128, :])
                nc.scalar.dma_start(xt_sb[fi], xt_d[fi * 128:(fi + 1) * 128, :])

            def load_w(name, dram, eng):
                t_ = cpool.tile([128, NF * EMB], BF16, name=name)
                chunks = [t_[:, ci * EMB:(ci + 1) * EMB] for ci in range(NF)]
                for ci in range(NF):
                    eng.dma_start(chunks[ci], dram[ci * 128:(ci + 1) * 128, :])
                return chunks

            wqt_sb = load_w("wqt_sb", wqt_d, nc.sync)
            bkq_sb = cpool.tile([128, 2 * NE], F32, name="bkq_sb")
            nc.sync.dma_start(bkq_sb[:], bkq_d[:])
            bkc_sb = bkq_sb[:, 0:NE]
            bqc_sb = bkq_sb[:, NE:2 * NE]
            m2_sb = cpool.tile([NSTART, HALF], BF16, name="m2_sb")
            nc.sync.dma_start(m2_sb[:], m2_d[:])
            m1_sb = cpool.tile([128, 128], BF16, name="m1_sb")
            nc.sync.dma_start(m1_sb[:], m1_d[:])
            ones_sb = cpool.tile([1, 128], F32R, name="ones_sb")
            nc.sync.dma_start(ones_sb[:], ones_d[:].bitcast(F32R))
            bv1r_sb = cpool.tile([1, EMB], F32R, name="bv1r_sb")
            nc.sync.dma_start(bv1r_sb[:], bv1_d[:].bitcast(F32R))
            bv2r_sb = cpool.tile([1, EMB], F32R, name="bv2r_sb")
            nc.sync.dma_start(bv2r_sb[:], bv2_d[:].bitcast(F32R))
            bur_sb = cpool.tile([1, EMB], F32R, name="bur_sb")
            nc.sync.dma_start(bur_sb[:], bu_d[:].bitcast(F32R))
            wvu1_sb = load_w("wvu1_sb", wvu1_d, nc.scalar)
            wvu2_sb = load_w("wvu2_sb", wvu2_d, nc.scalar)

            kt_sb = [bpool.tile([128, HALF], BF16, name=f"kt_sb{ei}")
                     for ei in range(NE)]
            for t0 in (0, 512):
                pss = [psum() for _ in range(NE)]
                for fi in range(NF):
                    for ei in range(NE):
                        nc.tensor.matmul(
                            pss[ei][:, :512],
                            wkt_sb[fi][:, ei * 128:(ei + 1) * 128],
                            xt_sb[fi][:, t0:t0 + 512],
                            start=(fi == 0), stop=(fi == NF - 1))
                for ei in range(NE):
                    nc.vector.tensor_scalar_add(
                        kt_sb[ei][:, t0:t0 + 512], pss[ei][:, :512],
                        bkc_sb[:, ei:ei + 1])

            bcast = {}
            for nm, src in (("bv1", bv1r_sb), ("bv2", bv2r_sb),
                            ("bu", bur_sb)):
                pb = psum()
                nc.tensor.matmul(pb[:, :EMB], ones_sb[:1, :], src[:1, :],
                                 start=True, stop=True)
                bb = cpool.tile([128, EMB], F32, name=f"{nm}b_sb")
                nc.vector.tensor_copy(bb[:], pb[:, :EMB])
                bcast[nm] = bb

            qt_sb = [bpool.tile([128, TOT], BF16, name=f"qt_sb{ei}")
                     for ei in range(NE)]
            spans = [(0, 512), (512, 512), (1024, NSTART)]
            for ei in range(NE):
                pss = [psum() for _ in spans]
                for fi in range(NF):
                    for si, (t0, sw) in enumerate(spans):
                        nc.tensor.matmul(
                            pss[si][:, :sw],
                            wqt_sb[fi][:, ei * 128:(ei + 1) * 128],
                            xt_sb[fi][:, t0:t0 + sw],
                            start=(fi == 0), stop=(fi == NF - 1))
                for si, (t0, sw) in enumerate(spans):
                    nc.vector.tensor_scalar_add(
                        qt_sb[ei][:, t0:t0 + sw], pss[si][:, :sw],
                        bqc_sb[:, ei:ei + 1])

            ps = psum()
            for fi in range(NF):
                nc.tensor.matmul(ps[:NSTART, :], xt_sb[fi][:, HALF:TOT],
                                 wvu2_sb[fi][:],
                                 start=(fi == 0), stop=(fi == NF - 1))
            v2s_sb = cpool.tile([NSTART, EMB], BF16, name="v2s_sb")
            nc.vector.tensor_add(v2s_sb[:], ps[:NSTART, :],
                                 bcast["bv2"][:NSTART, :])

            s2t_sb = bpool.tile([NSTART, HALF], BF16, name="s2t_sb")
            for t0 in (0, 512):
                ps2 = psum()
                for ei in range(NE):
                    nc.tensor.matmul(ps2[:NSTART, :512],
                                     qt_sb[ei][:, HALF:TOT],
                                     kt_sb[ei][:, t0:t0 + 512],
                                     start=(ei == 0), stop=(ei == NE - 1))
                nc.vector.tensor_mul(s2t_sb[:, t0:t0 + 512], ps2[:NSTART, :512],
                                     m2_sb[:, t0:t0 + 512])

            v1n_sb = [bpool.tile([128, EMB], BF16, name=f"v1n_sb{ti}")
                      for ti in range(NTI)]
            s1ts = [None] * NTI

            def emit_v1_s1(ti):
                t0 = ti * 128
                ps = psum()
                for fi in range(NF):
                    nc.tensor.matmul(ps[:, :], xt_sb[fi][:, t0:t0 + 128],
                                     wvu1_sb[fi][:],
                                     start=(fi == 0), stop=(fi == NF - 1))
                nc.vector.tensor_add(v1n_sb[ti][:], ps[:, :], bcast["bv1"][:])
                ps1 = psum()
                for ei in range(NE):
                    nc.tensor.matmul(ps1[:, :128],
                                     qt_sb[ei][:, t0:t0 + 128],
                                     kt_sb[ei][:, t0:t0 + 128],
                                     start=(ei == 0), stop=(ei == NE - 1))
                s1t = wpool.tile([128, 128], BF16, tag="s1t", name="s1t",
                                 bufs=4)
                nc.vector.tensor_mul(s1t[:], ps1[:, :128], m1_sb[:])
                s1ts[ti] = s1t

            def emit_av_out(ti):
                t0 = ti * 128
                ph = psum()
                nc.tensor.matmul(ph[:, :], s1ts[ti][:], v1n_sb[ti][:],
                                 start=True, stop=False)
                nc.tensor.matmul(ph[:, :], s2t_sb[:, t0:t0 + 128],
                                 v2s_sb[:], start=False, stop=True)
                ot = wpool.tile([128, EMB], F32, tag="ot", name="ot", bufs=3)
                nc.vector.tensor_add(ot[:], ph[:, :], bcast["bu"][:])
                nc.sync.dma_start(out_d[t0:t0 + 128, :], ot[:])

            for ti in range(NTI + 1):
                if ti < NTI:
                    emit_v1_s1(ti)
                if ti >= 1:
                    emit_av_out(ti - 1)

    return nc


_NC_CACHE = {}


def _get_program(with_bias):
    if with_bias not in _NC_CACHE:
        nc = build_bias() if with_bias else build_fast()
        nc.compile()          # bacc passes: wait splitting, reg alloc, ISA
        _NC_CACHE[with_bias] = nc
    return _NC_CACHE[with_bias]


def _make_masks():
    tri = np.triu(np.ones((KBLK, KBLK), np.float32))           # [c_l, r_l]
    m1 = np.kron(np.eye(2, dtype=np.float32), tri)             # [128, 128]
    # mask2[h][i, rl] = 1 if 64*i <= h*HALF + rl
    r = np.arange(HALF)
    m2 = []
    for h in range(2):
        blk = (h * HALF + r) // KBLK                           # [HALF]
        m2.append((np.arange(NSTART)[:, None] <= blk[None, :])
                  .astype(np.float32))
    return m1, m2


def make_in_maps(inputs, with_bias):
    x = np.asarray(inputs["x"], np.float32)
    wk = np.asarray(inputs["Wk"], np.float32)
    wq = np.asarray(inputs["Wq"], np.float32)
    wv = np.asarray(inputs["Wv"], np.float32)
    wu = np.asarray(inputs["Wu"], np.float32)
    bk = np.asarray(inputs["bk"], np.float32)
    bq = np.asarray(inputs["bq"], np.float32)
    bv = np.asarray(inputs["bv"], np.float32)
    bu = np.asarray(inputs["bu"], np.float32)

    # host-side weight preprocessing (pure weight-product folding)
    wvu1 = np.ascontiguousarray(wv.T @ wu[:, :EMB].T).astype(BF16NP)
    wvu2 = np.ascontiguousarray(wv.T @ wu[:, EMB:].T).astype(BF16NP)

    m1, m2 = _make_masks()
    starts = np.arange(NSTART) * KBLK

    in_maps = []
    for c in range(NCORES):
        b, h = c // 2, c % 2
        xin = np.concatenate(
            [x[b, h * HALF:(h + 1) * HALF], x[b, starts]], axis=0)
        m = {
            "xt": np.ascontiguousarray(xin.T.astype(BF16NP)),
            "wvu1": wvu1, "wvu2": wvu2,
            "mask1": m1.astype(BF16NP), "mask2": m2[h].astype(BF16NP),
        }
        if with_bias:
            m.update({
                "wkt": np.ascontiguousarray(wk.T).astype(BF16NP),
                "wqt": np.ascontiguousarray(wq.T).astype(BF16NP),
                "bkq": np.ascontiguousarray(np.concatenate(
                    [bk.reshape(EMB // 128, 128).T,
                     bq.reshape(EMB // 128, 128).T], axis=1)),
                "bv1r": (bv @ wu[:, :EMB].T).reshape(1, EMB).copy(),
                "bv2r": (bv @ wu[:, EMB:].T).reshape(1, EMB).copy(),
                "bur": bu.reshape(1, EMB).copy(),
                "ones": np.ones((1, 128), np.float32),
            })
        else:
            # At[f2, f1] = (Wk^T Wq)^T = Wq^T Wk
            m["at"] = np.ascontiguousarray(wq.T @ wk).astype(BF16NP)
        in_maps.append(m)
    return in_maps


def _ensure_ntff_hook():
    """The agent image lacks antenv.axon_hooks; synthesize it and register
    the ctypes NTFF profiling hook so trace=True works under axon."""
    import importlib.util
    if "antenv.axon_hooks" in sys.modules:
        return
    try:
        if importlib.util.find_spec("antenv.axon_hooks") is not None:
            return
    except ValueError:
        return
    import types
    import antenv
    m = types.ModuleType("antenv.axon_hooks")
    m._hook = None
    def set_axon_ntff_profile_hook(h):
        m._hook = h
    def get_axon_ntff_profile_hook():
        return m._hook
    m.set_axon_ntff_profile_hook = set_axon_ntff_profile_hook
    m.get_axon_ntff_profile_hook = get_axon_ntff_profile_hook
    sys.modules["antenv.axon_hooks"] = m
    antenv.axon_hooks = m
    try:
        from trn_agent_boot.trn_boot import _ntff_profile_via_ctypes
        m._hook = _ntff_profile_via_ctypes("/opt/axon/libaxon_pjrt.so")
    except Exception:
        pass


def run_sharded(inputs, trace=False, trace_kwargs=None):
    """inputs: dict of full numpy arrays keyed like setup_inputs().
    Returns (full_output [B, T, EMB] float32, BassKernelResults)."""
    if trace:
        _ensure_ntff_hook()
    with_bias = any(
        float(np.abs(np.asarray(inputs[k])).max()) != 0.0
        for k in ("bk", "bq", "bv", "bu"))
    in_maps = make_in_maps(inputs, with_bias)
    nc = _get_program(with_bias)
    res = run_bass_kernel_spmd(nc, in_maps, list(range(NCORES)), trace=trace,
                               **(trace_kwargs or {}))

    out = np.empty((B, T, EMB), np.float32)
    for c in range(NCORES):
        b, h = c // 2, c % 2
        out[b, h * HALF:(h + 1) * HALF] = res.results[c]["out"]
    return out, res


def kernel(**inputs):
    out, _ = run_sharded(inputs, trace=False)
    return out
